# revision 2
# baseline (speedup 1.0000x reference)
"""GAU attention (gated attention unit) Trainium2 Bass kernel.

Reference computation (B=2, S=2048, D=1024, H=16, DH=64):
    q = (hs @ Wq + bq), k = (hs @ Wk + bk), v = (hs @ Wv + bv)   per-head [B,S,H,DH]
    scores = q k^T / sqrt(DH);  probs = softmax(scores, axis=k)
    gating = sigmoid(gf * mean_d(hs) + gb)          # [B, S] per (batch, query)
    ctx = (probs * gating) @ v;  out = ctx @ Wo + bo

Sharding: 8 cores = 2 batches x 4 head-groups (4 heads each).  Each core
computes out^T partial [D, S] for its (batch, head-group); host sums the 4
partials per batch, applies the per-query gating scalar (gating commutes
through the linear AV + O-proj), and adds bo.

Per-core dataflow (all matmuls bf16 with fp32 PSUM accumulation):
  - hs^T [D,S] staged bf16 (host transposes + casts).
  - Q^T,K^T [256,S]: lhsT=W tiles (stationary), rhs=hs^T.  Layout: pair p of
    heads stacked on partitions (head A dh on 0:64, head B on 64:128).
  - V [S,256] natural: lhsT=hs^T tiles, rhs=Wv.
  - scores^T [k,q] per (pair, ktile, qchunk): two row-packed (tile_position
    (0,0)/(64,0)) K=64 matmuls -> exp on ACT (scale=1/8) -> E^T bf16.
  - softmax denom: E^T ktiles folded into two accumulators -- DVE takes
    tiles {0,1,7..15}, GPSIMD (otherwise idle) takes {2..6} -- combined on
    DVE, then ones[128,128] matmul broadcasts the partition reduction.
  - AV: col-packed (tile_position (0,0)/(0,64)) matmuls, V stationary,
    E^T streaming -> ctx^T accumulated over ktiles in PSUM.
  - ctx^T scaled by 1/denom (DVE, bf16 out), O-proj lhsT=Wo, rhs=ctx^T
    -> out^T bf16.
"""

import sys

for _p in ("/opt/trn_rl_repo", "/root/.axon_site/_ro/trn_rl_repo"):
    if _p not in sys.path:
        sys.path.append(_p)

from contextlib import ExitStack

import ml_dtypes
import numpy as np

import concourse.bass as bass
import concourse.mybir as mybir
import concourse.tile as tile
from concourse import bacc
from concourse.bass_utils import run_bass_kernel_spmd

BF16 = mybir.dt.bfloat16
F32 = mybir.dt.float32
AF = mybir.ActivationFunctionType
OP = mybir.AluOpType

B, S, D, H = 2, 2048, 1024, 16
DH = 64
HPC = 4  # heads per core
GD = HPC * DH  # 256 (head-group width)
NCORES = 8
NDT = D // 128  # 8 contraction tiles over D

GP_TILES = (2, 3, 4, 5, 6)  # E^T ktiles folded on GPSIMD (rest on DVE)


def _build(ctx: ExitStack, tc: "tile.TileContext", io: dict, s: int):
    nc = tc.nc
    GQ = min(512, s)
    NQC = s // GQ  # q chunks
    NKT = s // 128  # k tiles

    hsT, wq, wk, wv, wo = io["hsT"], io["wq"], io["wk"], io["wv"], io["wo"]
    bq, bk, bv, outT = io["bq"], io["bk"], io["bv"], io["outT"]

    consts = ctx.enter_context(tc.tile_pool(name="consts", bufs=1))
    sb = ctx.enter_context(tc.tile_pool(name="sb", bufs=1))
    etp = ctx.enter_context(tc.tile_pool(name="etp", bufs=6))
    ksp = ctx.enter_context(tc.tile_pool(name="ksp", bufs=2))
    outp = ctx.enter_context(tc.tile_pool(name="outp", bufs=4))
    # PSUM budget: 2x2 (scores, 2-bank tiles) + 2 (ctx) + 2 (denom/V-proj/
    # o-proj, shared tag) = 8 banks
    ps_mm = ctx.enter_context(tc.tile_pool(name="ps_mm", bufs=2, space="PSUM"))
    ps_ctx = ctx.enter_context(tc.tile_pool(name="ps_ctx", bufs=2, space="PSUM"))
    ps_o = ctx.enter_context(tc.tile_pool(name="ps_o", bufs=2, space="PSUM"))

    # ---- constants ----
    ones128 = consts.tile([128, 128], BF16, tag="ones128", name="ones128")
    nc.vector.memset(ones128[:], 1.0)

    bq_sb = consts.tile([128, 2], F32, tag="bq", name="bq")
    nc.sync.dma_start(bq_sb[:], bq.rearrange("(m p) -> p m", p=128))
    bk_sb = consts.tile([128, 2], F32, tag="bk", name="bk")
    nc.sync.dma_start(bk_sb[:], bk.rearrange("(m p) -> p m", p=128))
    # explicit zero bias for Exp, written by DVE so the wait merges with the
    # DVE wait the exps already carry
    zbias = consts.tile([128, 1], F32, tag="zbias", name="zbias")
    nc.vector.memset(zbias[:], 0.0)

    # bv arrives pre-broadcast [128, GD] from the host
    bv_bc = consts.tile([128, GD], F32, tag="bvbc", name="bvbc")
    nc.sync.dma_start(bv_bc[:], bv[:, :])

    # ---- load hs^T and weights ----
    hsT_sb = [sb.tile([128, s], BF16, tag=f"hsT{d}", name=f"hsT{d}") for d in range(NDT)]
    for qc in range(NQC):
        cs = slice(qc * GQ, (qc + 1) * GQ)
        for d in range(NDT):
            nc.sync.dma_start(hsT_sb[d][:, cs], hsT[d * 128 : (d + 1) * 128, cs])

    wq_sb = [consts.tile([128, GD], BF16, tag=f"wq{d}", name=f"wq{d}") for d in range(NDT)]
    wk_sb = [consts.tile([128, GD], BF16, tag=f"wk{d}", name=f"wk{d}") for d in range(NDT)]
    wv_sb = [consts.tile([128, GD], BF16, tag=f"wv{d}", name=f"wv{d}") for d in range(NDT)]
    for d in range(NDT):
        rs = slice(d * 128, (d + 1) * 128)
        nc.sync.dma_start(wq_sb[d][:], wq[rs, :])
        nc.sync.dma_start(wk_sb[d][:], wk[rs, :])
        nc.sync.dma_start(wv_sb[d][:], wv[rs, :])
    wo_sb = [consts.tile([128, D], BF16, tag=f"wo{p}", name=f"wo{p}") for p in range(2)]
    for p in range(2):
        nc.sync.dma_start(wo_sb[p][:], wo[p * 128 : (p + 1) * 128, :])

    # ---- K^T projection: [256, s] as 2 pair-tiles [128, s] bf16 ----
    # (K and V are needed in full before any attention; Q is computed
    # per q-chunk inside the attention loop so exps start early)
    qT_sb = [sb.tile([128, s], BF16, tag=f"qT{m}", name=f"qT{m}") for m in range(2)]
    kT_sb = [sb.tile([128, s], BF16, tag=f"kT{m}", name=f"kT{m}") for m in range(2)]
    for m in range(2):
        ms = slice(m * 128, (m + 1) * 128)
        for qc in range(NQC):
            cs = slice(qc * GQ, (qc + 1) * GQ)
            p = ps_mm.tile([128, GQ], F32, tag="smm", name="smm")
            for d in range(NDT):
                nc.tensor.matmul(
                    p[:], lhsT=wk_sb[d][:, ms], rhs=hsT_sb[d][:, cs],
                    start=(d == 0), stop=(d == NDT - 1),
                )
            nc.vector.tensor_scalar_add(kT_sb[m][:, cs], p[:], bk_sb[:, m : m + 1])

    # V tiles are projected lazily, interleaved into qc0's attention groups
    v_sb = [sb.tile([128, GD], BF16, tag=f"v{st}", name=f"v{st}") for st in range(NKT)]

    # ---- per q-chunk: Q-proj, attention, output projection ----
    for qc in range(NQC):
        cs = slice(qc * GQ, (qc + 1) * GQ)

        # Q^T projection for this chunk
        for m in range(2):
            ms = slice(m * 128, (m + 1) * 128)
            p = ps_mm.tile([128, GQ], F32, tag="smm", name="smm")
            for d in range(NDT):
                nc.tensor.matmul(
                    p[:], lhsT=wq_sb[d][:, ms], rhs=hsT_sb[d][:, cs],
                    start=(d == 0), stop=(d == NDT - 1),
                )
            nc.vector.tensor_scalar_add(qT_sb[m][:, cs], p[:], bq_sb[:, m : m + 1])

        ctx_sc = [sb.tile([128, GQ], BF16, tag=f"ctxs{p}_{qc % 2}", name=f"ctxs{p}_{qc % 2}") for p in range(2)]
        for pr in range(2):
            # one PSUM bank per head: col-packed AV matmuls write disjoint
            # partition ranges, but each bank hosts a single accumulation group
            ctx_psA = ps_ctx.tile([128, GQ], F32, tag="ctx", name="ctxA")
            ctx_psB = ps_ctx.tile([128, GQ], F32, tag="ctx", name="ctxB")
            ks_dve = ksp.tile([128, 2 * GQ], BF16, tag="ksdve", name="ksdve")
            ks_gp = ksp.tile([128, 2 * GQ], BF16, tag="ksgp", name="ksgp")
            # Software-pipelined over kt: the AV pair for kt-1 is emitted right
            # after the scores pair for kt, so the four matmuls sit adjacently
            # in the PE stream.  exp/fold of kt overlap the next group's
            # matmuls.  E^T fold work is split DVE/GPSIMD (two accumulators).
            ets = [None] * NKT
            for kt in range(NKT + 1):
                if kt < NKT:
                    ks_ = slice(kt * 128, (kt + 1) * 128)
                    # heads A|B share one 2-bank psum tile -> single wide exp
                    sp = ps_mm.tile([128, 2 * GQ], F32, tag="smm", name="smm")
                    nc.tensor.matmul(
                        sp[:, 0:GQ], lhsT=kT_sb[pr][0:64, ks_], rhs=qT_sb[pr][0:64, cs],
                        tile_position=(0, 0), start=True, stop=True,
                    )
                    nc.tensor.matmul(
                        sp[:, GQ : 2 * GQ], lhsT=kT_sb[pr][64:128, ks_], rhs=qT_sb[pr][64:128, cs],
                        tile_position=(64, 0), start=True, stop=True,
                    )
                    if qc == 0 and pr == 0:
                        # V projection for tile kt, just in time for its AV
                        # matmul one group later (ps_o is idle during the loop)
                        ss = slice(kt * 128, (kt + 1) * 128)
                        vp = ps_o.tile([128, GD], F32, tag="po", name="vp")
                        for d in range(NDT):
                            nc.tensor.matmul(
                                vp[:], lhsT=hsT_sb[d][:, ss], rhs=wv_sb[d][:],
                                start=(d == 0), stop=(d == NDT - 1),
                            )
                        nc.vector.tensor_tensor(v_sb[kt][:], vp[:], bv_bc[:], op=OP.add)
                if kt > 0:
                    pv = kt - 1
                    et = ets[pv]
                    nc.tensor.matmul(
                        ctx_psA[0:64, :], lhsT=v_sb[pv][:, pr * 128 : pr * 128 + 64],
                        rhs=et[:, 0:GQ], tile_position=(0, 0),
                        start=(pv == 0), stop=(pv == NKT - 1),
                    )
                    nc.tensor.matmul(
                        ctx_psB[64:128, :], lhsT=v_sb[pv][:, pr * 128 + 64 : pr * 128 + 128],
                        rhs=et[:, GQ : 2 * GQ], tile_position=(0, 64),
                        start=(pv == 0), stop=(pv == NKT - 1),
                    )
                if kt < NKT:
                    et = etp.tile([128, 2 * GQ], BF16, tag="et", name="et")
                    ets[kt] = et
                    # single wide exp over both heads (2-bank PSUM read)
                    nc.scalar.activation(et[:], sp[:], AF.Exp, bias=zbias[:, 0:1], scale=0.125)
                    # fold E^T into the denominator accumulators as tiles land
                    if kt == 1:
                        nc.vector.tensor_tensor(ks_dve[:], ets[0][:], ets[1][:], op=OP.add)
                    elif kt == GP_TILES[1]:
                        nc.gpsimd.tensor_tensor(
                            ks_gp[:], ets[GP_TILES[0]][:], ets[GP_TILES[1]][:], op=OP.add
                        )
                    elif kt in GP_TILES:
                        nc.gpsimd.tensor_tensor(ks_gp[:], ks_gp[:], et[:], op=OP.add)
                    elif kt > 1:
                        nc.vector.tensor_tensor(ks_dve[:], ks_dve[:], et[:], op=OP.add)
            # combine the two fold accumulators
            nc.vector.tensor_tensor(ks_dve[:], ks_dve[:], ks_gp[:], op=OP.add)
            # softmax denominators (broadcast to all partitions) and ctx scaling
            rA = ksp.tile([128, GQ], F32, tag="rrA", name="rrA")
            rB = ksp.tile([128, GQ], F32, tag="rrB", name="rrB")
            for hh, r_sb in ((0, rA), (1, rB)):
                db_ps = ps_o.tile([128, GQ], F32, tag="po", name="po")
                nc.tensor.matmul(
                    db_ps[:], lhsT=ones128[:], rhs=ks_dve[:, hh * GQ : (hh + 1) * GQ],
                    start=True, stop=True,
                )
                nc.vector.reciprocal_approx_fast(r_sb[:], db_ps[:])
            nc.vector.tensor_tensor(ctx_sc[pr][0:64, :], ctx_psA[0:64, :], rA[0:64, :], op=OP.mult)
            nc.vector.tensor_tensor(ctx_sc[pr][64:128, :], ctx_psB[64:128, :], rB[64:128, :], op=OP.mult)

        # output projection: out^T[mt*128:(mt+1)*128, cs] = Wo^T @ ctx^T
        for mt in range(D // 128):
            ms = slice(mt * 128, (mt + 1) * 128)
            o_ps = ps_o.tile([128, GQ], F32, tag="po", name="po")
            for pr in range(2):
                nc.tensor.matmul(
                    o_ps[:], lhsT=wo_sb[pr][:, ms], rhs=ctx_sc[pr][:],
                    start=(pr == 0), stop=(pr == 1),
                )
            ost = outp.tile([128, GQ], BF16, tag="ost", name="ost")
            nc.vector.tensor_copy(ost[:], o_ps[:])
            nc.sync.dma_start(outT[ms, cs], ost[:])


def build_gau_nc(s: int = S, debug: bool = False):
    nc = bacc.Bacc("TRN2", target_bir_lowering=False, debug=debug, num_devices=NCORES)
    io = {
        "hsT": nc.dram_tensor("hsT", [D, s], BF16, kind="ExternalInput").ap(),
        "wq": nc.dram_tensor("wq", [D, GD], BF16, kind="ExternalInput").ap(),
        "wk": nc.dram_tensor("wk", [D, GD], BF16, kind="ExternalInput").ap(),
        "wv": nc.dram_tensor("wv", [D, GD], BF16, kind="ExternalInput").ap(),
        "wo": nc.dram_tensor("wo", [GD, D], BF16, kind="ExternalInput").ap(),
        "bq": nc.dram_tensor("bq", [GD], F32, kind="ExternalInput").ap(),
        "bk": nc.dram_tensor("bk", [GD], F32, kind="ExternalInput").ap(),
        "bv": nc.dram_tensor("bv", [128, GD], F32, kind="ExternalInput").ap(),
        "outT": nc.dram_tensor("outT", [D, s], BF16, kind="ExternalOutput").ap(),
    }
    with tile.TileContext(nc) as tc:
        with ExitStack() as ctx:
            _build(ctx, tc, io, s)
    nc.compile()
    return nc


def make_in_maps(hidden_states, Wq, bq, Wk, bk, Wv, bv, Wo, gating_factor, gating_bias):
    """Shard full inputs into 8 per-core input maps (host-side prep)."""
    bf = ml_dtypes.bfloat16
    f32 = np.float32
    hs = np.asarray(hidden_states, f32)
    Wq, Wk, Wv, Wo = (np.asarray(a, f32) for a in (Wq, Wk, Wv, Wo))
    bq, bk, bv = (np.asarray(a, f32) for a in (bq, bk, bv))

    hsT_b = [np.ascontiguousarray(hs[b].T).astype(bf) for b in range(B)]
    in_maps = []
    for c in range(NCORES):
        b, g = divmod(c, NCORES // B)
        cols = slice(g * GD, (g + 1) * GD)
        in_maps.append(
            {
                "hsT": hsT_b[b],
                "wq": np.ascontiguousarray(Wq[:, cols]).astype(bf),
                "wk": np.ascontiguousarray(Wk[:, cols]).astype(bf),
                "wv": np.ascontiguousarray(Wv[:, cols]).astype(bf),
                "wo": np.ascontiguousarray(Wo[cols, :]).astype(bf),
                "bq": np.ascontiguousarray(bq[cols]),
                "bk": np.ascontiguousarray(bk[cols]),
                "bv": np.ascontiguousarray(np.broadcast_to(bv[cols], (128, GD))),
            }
        )
    return in_maps


_NC_CACHE: dict = {}


def _get_nc(s: int = S):
    if s not in _NC_CACHE:
        _NC_CACHE[s] = build_gau_nc(s)
    return _NC_CACHE[s]


def run_gau(in_maps, **kwargs):
    nc = _get_nc(S)
    return run_bass_kernel_spmd(nc, in_maps, core_ids=list(range(NCORES)), **kwargs)


def assemble_output(results, hidden_states, gating_factor, gating_bias, bo):
    """Sum per-batch head-group partials, transpose back, apply gating + bo.

    The per-(batch, query) gating scalar commutes through the AV matmul and
    the output projection, so it is applied here on the host.
    """
    bo = np.asarray(bo, np.float32)
    hs = np.asarray(hidden_states, np.float32)
    gf = np.float32(np.asarray(gating_factor, np.float32)[0])
    gb = np.float32(np.asarray(gating_bias, np.float32)[0])
    gate = 1.0 / (1.0 + np.exp(-(gf * hs.mean(axis=-1) + gb)))  # [B, S]
    gpb = NCORES // B
    out = np.empty((B, S, D), np.float32)
    for b in range(B):
        acc = results[gpb * b]["outT"].astype(np.float32)
        for g in range(1, gpb):
            acc = acc + results[gpb * b + g]["outT"].astype(np.float32)
        out[b] = acc.T * gate[b][:, None] + bo[None, :]
    return out


def kernel(hidden_states, Wq, bq, Wk, bk, Wv, bv, Wo, bo, gating_factor, gating_bias):
    in_maps = make_in_maps(
        hidden_states, Wq, bq, Wk, bk, Wv, bv, Wo, gating_factor, gating_bias
    )
    res = run_gau(in_maps)
    return assemble_output(res.results, hidden_states, gating_factor, gating_bias, bo)


# revision 3
# speedup vs baseline: 1.0297x; 1.0297x over previous
"""GAU attention (gated attention unit) Trainium2 Bass kernel.

Reference computation (B=2, S=2048, D=1024, H=16, DH=64):
    q = (hs @ Wq + bq), k = (hs @ Wk + bk), v = (hs @ Wv + bv)   per-head [B,S,H,DH]
    scores = q k^T / sqrt(DH);  probs = softmax(scores, axis=k)
    gating = sigmoid(gf * mean_d(hs) + gb)          # [B, S] per (batch, query)
    ctx = (probs * gating) @ v;  out = ctx @ Wo + bo

Sharding: 8 cores = 2 batches x 4 head-groups (4 heads each).  Each core
computes out^T partial [D, S] for its (batch, head-group); host sums the 4
partials per batch, applies the per-query gating scalar (gating commutes
through the linear AV + O-proj), and adds bo.

Per-core dataflow (matmuls bf16 with fp32 PSUM accumulation).  The kernel is
ACT(exp)-bound in steady state, so all projection work is interleaved into
the attention kt-loops to keep the exp pipeline dense:
  - hs^T [D,S] staged bf16 (host transposes + casts).
  - K^T [256,S]: chunk 0 of head-pair 0 projected up front; remaining chunks
    just-in-time inside the (qc0, pr) kt-loops.
  - V [S,256]: projected just-in-time inside the (qc0, pr0) kt-loop.
  - scores^T [k,q] per (pair, ktile, qchunk): two row-packed (tile_position
    (0,0)/(64,0)) K=64 matmuls -> exp -> E^T bf16.  Most tiles exp on ACT
    (scale=1/8); tiles in OFFLOAD_KT use a clamped bf16 exp bit-trick on DVE
    (two tensor_scalar ops) to relieve the ACT bottleneck.
  - softmax denom: DVE folds E^T ktiles into ks (first fold sums tiles 0+1),
    then ones[128,128] matmul broadcasts the 128-partition reduction.
  - AV: col-packed (tile_position (0,0)/(0,64)) matmuls, V stationary,
    E^T streaming -> ctx^T accumulated over ktiles in PSUM.
  - ctx^T scaled by 1/denom (DVE, bf16 out).  O-proj of chunk qc is
    interleaved into chunk qc+1's pr0 kt-loop; Q-proj of qc+1 into qc's pr1
    kt-loop, so ACT never waits on projection phases.
"""

import sys

for _p in ("/opt/trn_rl_repo", "/root/.axon_site/_ro/trn_rl_repo"):
    if _p not in sys.path:
        sys.path.append(_p)

import math
from contextlib import ExitStack

import ml_dtypes
import numpy as np

import concourse.bass as bass
import concourse.mybir as mybir
import concourse.tile as tile
from concourse import bacc
from concourse.bass_utils import run_bass_kernel_spmd

BF16 = mybir.dt.bfloat16
F32 = mybir.dt.float32
I16 = mybir.dt.int16
AF = mybir.ActivationFunctionType
OP = mybir.AluOpType

B, S, D, H = 2, 2048, 1024, 16
DH = 64
HPC = 4  # heads per core
GD = HPC * DH  # 256 (head-group width)
NCORES = 8
NDT = D // 128  # 8 contraction tiles over D

# E^T ktiles whose exp runs on DVE (bit-trick) instead of ACT
OFFLOAD_KT = (5, 11)
# bf16 Schraudolph constants: i16 = min(max(s*EXA, EXLO), EXHI) + EXB;
# E = bitcast_bf16(i16) ~= exp(s/8), max rel err ~4%, zero-mean.
EXA = 0.125 * 128.0 / math.log(2.0)
EXB = 16256.0 - 486411.0 / 65536.0 + 0.5
EXLO = 128.0 - 16249.0
EXHI = 18304.0 - 16249.0


def _build(ctx: ExitStack, tc: "tile.TileContext", io: dict, s: int):
    nc = tc.nc
    GQ = min(512, s)
    NQC = s // GQ  # q chunks
    NKT = s // 128  # k tiles

    hsT, wq, wk, wv, wo = io["hsT"], io["wq"], io["wk"], io["wv"], io["wo"]
    bq, bk, bv, outT = io["bq"], io["bk"], io["bv"], io["outT"]

    consts = ctx.enter_context(tc.tile_pool(name="consts", bufs=1))
    sb = ctx.enter_context(tc.tile_pool(name="sb", bufs=1))
    etp = ctx.enter_context(tc.tile_pool(name="etp", bufs=6))
    ksp = ctx.enter_context(tc.tile_pool(name="ksp", bufs=2))
    outp = ctx.enter_context(tc.tile_pool(name="outp", bufs=4))
    # PSUM budget: 2x2 (scores, 2-bank tiles) + 2 (ctx) + 2 (proj/denom/
    # o-proj, shared tag) = 8 banks
    ps_mm = ctx.enter_context(tc.tile_pool(name="ps_mm", bufs=2, space="PSUM"))
    ps_ctx = ctx.enter_context(tc.tile_pool(name="ps_ctx", bufs=2, space="PSUM"))
    ps_o = ctx.enter_context(tc.tile_pool(name="ps_o", bufs=2, space="PSUM"))

    # ---- constants ----
    ones128 = consts.tile([128, 128], BF16, tag="ones128", name="ones128")
    nc.vector.memset(ones128[:], 1.0)

    bq_sb = consts.tile([128, 2], F32, tag="bq", name="bq")
    nc.sync.dma_start(bq_sb[:], bq.rearrange("(m p) -> p m", p=128))
    bk_sb = consts.tile([128, 2], F32, tag="bk", name="bk")
    nc.sync.dma_start(bk_sb[:], bk.rearrange("(m p) -> p m", p=128))
    zbias = consts.tile([128, 1], F32, tag="zbias", name="zbias")
    nc.vector.memset(zbias[:], 0.0)
    # bv arrives pre-broadcast [128, GD] from the host
    bv_bc = consts.tile([128, GD], F32, tag="bvbc", name="bvbc")
    nc.sync.dma_start(bv_bc[:], bv[:, :])

    # ---- load hs^T and weights ----
    hsT_sb = [sb.tile([128, s], BF16, tag=f"hsT{d}", name=f"hsT{d}") for d in range(NDT)]
    for qc in range(NQC):
        cs = slice(qc * GQ, (qc + 1) * GQ)
        for d in range(NDT):
            nc.sync.dma_start(hsT_sb[d][:, cs], hsT[d * 128 : (d + 1) * 128, cs])

    wq_sb = [consts.tile([128, GD], BF16, tag=f"wq{d}", name=f"wq{d}") for d in range(NDT)]
    wk_sb = [consts.tile([128, GD], BF16, tag=f"wk{d}", name=f"wk{d}") for d in range(NDT)]
    wv_sb = [consts.tile([128, GD], BF16, tag=f"wv{d}", name=f"wv{d}") for d in range(NDT)]
    for d in range(NDT):
        rs = slice(d * 128, (d + 1) * 128)
        nc.sync.dma_start(wq_sb[d][:], wq[rs, :])
        nc.sync.dma_start(wk_sb[d][:], wk[rs, :])
        nc.sync.dma_start(wv_sb[d][:], wv[rs, :])
    wo_sb = [consts.tile([128, D], BF16, tag=f"wo{p}", name=f"wo{p}") for p in range(2)]
    for p in range(2):
        nc.sync.dma_start(wo_sb[p][:], wo[p * 128 : (p + 1) * 128, :])

    qT_sb = [sb.tile([128, s], BF16, tag=f"qT{m}", name=f"qT{m}") for m in range(2)]
    kT_sb = [sb.tile([128, s], BF16, tag=f"kT{m}", name=f"kT{m}") for m in range(2)]
    v_sb = [sb.tile([128, GD], BF16, tag=f"v{st}", name=f"v{st}") for st in range(NKT)]

    def kproj(m, c):
        ms = slice(m * 128, (m + 1) * 128)
        cls = slice(c * GQ, (c + 1) * GQ)
        p = ps_o.tile([128, GQ], F32, tag="po", name="kp")
        for d in range(NDT):
            nc.tensor.matmul(
                p[:], lhsT=wk_sb[d][:, ms], rhs=hsT_sb[d][:, cls],
                start=(d == 0), stop=(d == NDT - 1),
            )
        nc.vector.tensor_scalar_add(kT_sb[m][:, cls], p[:], bk_sb[:, m : m + 1])

    def qproj(m, c):
        ms = slice(m * 128, (m + 1) * 128)
        cls = slice(c * GQ, (c + 1) * GQ)
        p = ps_o.tile([128, GQ], F32, tag="po", name="qp")
        for d in range(NDT):
            nc.tensor.matmul(
                p[:], lhsT=wq_sb[d][:, ms], rhs=hsT_sb[d][:, cls],
                start=(d == 0), stop=(d == NDT - 1),
            )
        nc.vector.tensor_scalar_add(qT_sb[m][:, cls], p[:], bq_sb[:, m : m + 1])

    def vproj(st):
        ss = slice(st * 128, (st + 1) * 128)
        vp = ps_o.tile([128, GD], F32, tag="po", name="vp")
        for d in range(NDT):
            nc.tensor.matmul(
                vp[:], lhsT=hsT_sb[d][:, ss], rhs=wv_sb[d][:],
                start=(d == 0), stop=(d == NDT - 1),
            )
        nc.vector.tensor_tensor(v_sb[st][:], vp[:], bv_bc[:], op=OP.add)

    ctx_sc_of = {}

    def oproj(qc, mt):
        cs = slice(qc * GQ, (qc + 1) * GQ)
        ms = slice(mt * 128, (mt + 1) * 128)
        o_ps = ps_o.tile([128, GQ], F32, tag="po", name="po")
        for pr in range(2):
            nc.tensor.matmul(
                o_ps[:], lhsT=wo_sb[pr][:, ms], rhs=ctx_sc_of[qc][pr][:],
                start=(pr == 0), stop=(pr == 1),
            )
        ost = outp.tile([128, GQ], BF16, tag="ost", name="ost")
        nc.vector.tensor_copy(ost[:], o_ps[:])
        nc.sync.dma_start(outT[ms, cs], ost[:])

    # serial head: K^T chunk 0 (pair 0) and Q^T chunk 0, so scores start ASAP
    kproj(0, 0)
    qproj(0, 0)
    qproj(1, 0)

    # interleave schedule: (qc, pr) -> {kt: [work closures]}
    def make_sched():
        sched = {(qc, pr): {} for qc in range(NQC) for pr in range(2)}

        def add(qc, pr, kt, fn):
            sched[(qc, pr)].setdefault(kt, []).append(fn)

        for st in range(NKT):  # V just-in-time in (qc0, pr0)
            add(0, 0, st, lambda st=st: vproj(st))
        for c in range(1, NQC):  # K chunks 1..3, pair 0
            add(0, 0, 4 * c - 3, lambda c=c: kproj(0, c))
        add(0, 0, NKT - 3, lambda: kproj(1, 0))  # K chunk 0, pair 1
        for c in range(1, NQC):  # K chunks 1..3, pair 1
            add(0, 1, 4 * c - 3, lambda c=c: kproj(1, c))
        for qc in range(NQC - 1):  # Q-proj of qc+1 late in (qc, pr1)
            add(qc, 1, NKT - 5, lambda qc=qc: qproj(0, qc + 1))
            add(qc, 1, NKT - 3, lambda qc=qc: qproj(1, qc + 1))
        for qc in range(1, NQC):  # O-proj of qc-1 inside (qc, pr0)
            for mt in range(D // 128):
                add(qc, 0, mt + 1, lambda qc=qc, mt=mt: oproj(qc - 1, mt))
        return sched

    sched = make_sched()

    # ---- per q-chunk: attention (projections interleaved per sched) ----
    for qc in range(NQC):
        cs = slice(qc * GQ, (qc + 1) * GQ)
        ctx_sc = [sb.tile([128, GQ], BF16, tag=f"ctxs{p}_{qc % 2}", name=f"ctxs{p}_{qc % 2}") for p in range(2)]
        ctx_sc_of[qc] = ctx_sc
        for pr in range(2):
            # one PSUM bank per head: col-packed AV matmuls write disjoint
            # partition ranges, but each bank hosts a single accumulation group
            ctx_psA = ps_ctx.tile([128, GQ], F32, tag="ctx", name="ctxA")
            ctx_psB = ps_ctx.tile([128, GQ], F32, tag="ctx", name="ctxB")
            ks = ksp.tile([128, 2 * GQ], BF16, tag="ks", name="ks")
            # Software-pipelined over kt: the AV pair for kt-1 is emitted right
            # after the scores pair for kt; exp/fold of kt overlap the next
            # group's matmuls; interleaved projection work fills PE slack.
            ets = [None] * NKT
            for kt in range(NKT + 1):
                if kt < NKT:
                    ks_ = slice(kt * 128, (kt + 1) * 128)
                    # heads A|B share one 2-bank psum tile -> single wide exp
                    sp = ps_mm.tile([128, 2 * GQ], F32, tag="smm", name="smm")
                    nc.tensor.matmul(
                        sp[:, 0:GQ], lhsT=kT_sb[pr][0:64, ks_], rhs=qT_sb[pr][0:64, cs],
                        tile_position=(0, 0), start=True, stop=True,
                    )
                    nc.tensor.matmul(
                        sp[:, GQ : 2 * GQ], lhsT=kT_sb[pr][64:128, ks_], rhs=qT_sb[pr][64:128, cs],
                        tile_position=(64, 0), start=True, stop=True,
                    )
                    for fn in sched[(qc, pr)].get(kt, ()):
                        fn()
                if kt > 0:
                    pv = kt - 1
                    et = ets[pv]
                    nc.tensor.matmul(
                        ctx_psA[0:64, :], lhsT=v_sb[pv][:, pr * 128 : pr * 128 + 64],
                        rhs=et[:, 0:GQ], tile_position=(0, 0),
                        start=(pv == 0), stop=(pv == NKT - 1),
                    )
                    nc.tensor.matmul(
                        ctx_psB[64:128, :], lhsT=v_sb[pv][:, pr * 128 + 64 : pr * 128 + 128],
                        rhs=et[:, GQ : 2 * GQ], tile_position=(0, 64),
                        start=(pv == 0), stop=(pv == NKT - 1),
                    )
                if kt < NKT:
                    et = etp.tile([128, 2 * GQ], BF16, tag="et", name="et")
                    ets[kt] = et
                    if kt in OFFLOAD_KT:
                        # clamped bf16 exp bit-trick on DVE (2 tensor_scalar ops)
                        u = ksp.tile([128, 2 * GQ], F32, tag="shru", name="shru")
                        nc.vector.tensor_scalar(u[:], sp[:], EXA, EXLO, op0=OP.mult, op1=OP.max)
                        nc.vector.tensor_scalar(
                            et[:].bitcast(I16), u[:], EXHI, EXB, op0=OP.min, op1=OP.add
                        )
                    else:
                        # single wide exp over both heads (2-bank PSUM read)
                        nc.scalar.activation(et[:], sp[:], AF.Exp, bias=zbias[:, 0:1], scale=0.125)
                    if kt == 1:
                        nc.vector.tensor_tensor(ks[:], ets[0][:], ets[1][:], op=OP.add)
                    elif kt > 1:
                        nc.vector.tensor_tensor(ks[:], ks[:], et[:], op=OP.add)
            # softmax denominators (broadcast to all partitions) and ctx scaling
            rA = ksp.tile([128, GQ], F32, tag="rrA", name="rrA")
            rB = ksp.tile([128, GQ], F32, tag="rrB", name="rrB")
            for hh, r_sb in ((0, rA), (1, rB)):
                db_ps = ps_o.tile([128, GQ], F32, tag="po", name="po")
                nc.tensor.matmul(
                    db_ps[:], lhsT=ones128[:], rhs=ks[:, hh * GQ : (hh + 1) * GQ],
                    start=True, stop=True,
                )
                nc.vector.reciprocal_approx_fast(r_sb[:], db_ps[:])
            nc.vector.tensor_tensor(ctx_sc[pr][0:64, :], ctx_psA[0:64, :], rA[0:64, :], op=OP.mult)
            nc.vector.tensor_tensor(ctx_sc[pr][64:128, :], ctx_psB[64:128, :], rB[64:128, :], op=OP.mult)

    # tail: O-projection of the last chunk
    for mt in range(D // 128):
        oproj(NQC - 1, mt)


def build_gau_nc(s: int = S, debug: bool = False):
    nc = bacc.Bacc("TRN2", target_bir_lowering=False, debug=debug, num_devices=NCORES)
    io = {
        "hsT": nc.dram_tensor("hsT", [D, s], BF16, kind="ExternalInput").ap(),
        "wq": nc.dram_tensor("wq", [D, GD], BF16, kind="ExternalInput").ap(),
        "wk": nc.dram_tensor("wk", [D, GD], BF16, kind="ExternalInput").ap(),
        "wv": nc.dram_tensor("wv", [D, GD], BF16, kind="ExternalInput").ap(),
        "wo": nc.dram_tensor("wo", [GD, D], BF16, kind="ExternalInput").ap(),
        "bq": nc.dram_tensor("bq", [GD], F32, kind="ExternalInput").ap(),
        "bk": nc.dram_tensor("bk", [GD], F32, kind="ExternalInput").ap(),
        "bv": nc.dram_tensor("bv", [128, GD], F32, kind="ExternalInput").ap(),
        "outT": nc.dram_tensor("outT", [D, s], BF16, kind="ExternalOutput").ap(),
    }
    with tile.TileContext(nc) as tc:
        with ExitStack() as ctx:
            _build(ctx, tc, io, s)
    nc.compile()
    return nc


def make_in_maps(hidden_states, Wq, bq, Wk, bk, Wv, bv, Wo, gating_factor, gating_bias):
    """Shard full inputs into 8 per-core input maps (host-side prep)."""
    bf = ml_dtypes.bfloat16
    f32 = np.float32
    hs = np.asarray(hidden_states, f32)
    Wq, Wk, Wv, Wo = (np.asarray(a, f32) for a in (Wq, Wk, Wv, Wo))
    bq, bk, bv = (np.asarray(a, f32) for a in (bq, bk, bv))

    hsT_b = [np.ascontiguousarray(hs[b].T).astype(bf) for b in range(B)]
    in_maps = []
    for c in range(NCORES):
        b, g = divmod(c, NCORES // B)
        cols = slice(g * GD, (g + 1) * GD)
        in_maps.append(
            {
                "hsT": hsT_b[b],
                "wq": np.ascontiguousarray(Wq[:, cols]).astype(bf),
                "wk": np.ascontiguousarray(Wk[:, cols]).astype(bf),
                "wv": np.ascontiguousarray(Wv[:, cols]).astype(bf),
                "wo": np.ascontiguousarray(Wo[cols, :]).astype(bf),
                "bq": np.ascontiguousarray(bq[cols]),
                "bk": np.ascontiguousarray(bk[cols]),
                "bv": np.ascontiguousarray(np.broadcast_to(bv[cols], (128, GD))),
            }
        )
    return in_maps


_NC_CACHE: dict = {}


def _get_nc(s: int = S):
    if s not in _NC_CACHE:
        _NC_CACHE[s] = build_gau_nc(s)
    return _NC_CACHE[s]


def run_gau(in_maps, **kwargs):
    nc = _get_nc(S)
    return run_bass_kernel_spmd(nc, in_maps, core_ids=list(range(NCORES)), **kwargs)


def assemble_output(results, hidden_states, gating_factor, gating_bias, bo):
    """Sum per-batch head-group partials, transpose back, apply gating + bo.

    The per-(batch, query) gating scalar commutes through the AV matmul and
    the output projection, so it is applied here on the host.
    """
    bo = np.asarray(bo, np.float32)
    hs = np.asarray(hidden_states, np.float32)
    gf = np.float32(np.asarray(gating_factor, np.float32)[0])
    gb = np.float32(np.asarray(gating_bias, np.float32)[0])
    gate = 1.0 / (1.0 + np.exp(-(gf * hs.mean(axis=-1) + gb)))  # [B, S]
    gpb = NCORES // B
    out = np.empty((B, S, D), np.float32)
    for b in range(B):
        acc = results[gpb * b]["outT"].astype(np.float32)
        for g in range(1, gpb):
            acc = acc + results[gpb * b + g]["outT"].astype(np.float32)
        out[b] = acc.T * gate[b][:, None] + bo[None, :]
    return out


def kernel(hidden_states, Wq, bq, Wk, bk, Wv, bv, Wo, bo, gating_factor, gating_bias):
    in_maps = make_in_maps(
        hidden_states, Wq, bq, Wk, bk, Wv, bv, Wo, gating_factor, gating_bias
    )
    res = run_gau(in_maps)
    return assemble_output(res.results, hidden_states, gating_factor, gating_bias, bo)


# revision 6
# speedup vs baseline: 1.1698x; 1.1361x over previous
"""GAU attention (gated attention unit) Trainium2 Bass kernel.

Reference computation (B=2, S=2048, D=1024, H=16, DH=64):
    q = (hs @ Wq + bq), k = (hs @ Wk + bk), v = (hs @ Wv + bv)   per-head [B,S,H,DH]
    scores = q k^T / sqrt(DH);  probs = softmax(scores, axis=k)
    gating = sigmoid(gf * mean_d(hs) + gb)          # [B, S] per (batch, query)
    ctx = (probs * gating) @ v;  out = ctx @ Wo + bo

Sharding: 8 cores = 2 batches x 4 head-groups (4 heads each).  Each core
computes out^T partial [D, S] for its (batch, head-group); host sums the 4
partials per batch, applies the per-query gating scalar (gating commutes
through the linear AV + O-proj), and adds bo.

Per-core dataflow (matmuls bf16 with fp32 PSUM accumulation).  The kernel is
ACT(exp)-bound in steady state, so all projection work is interleaved into
the attention kt-loops to keep the exp pipeline dense:
  - hs^T [D,S] staged bf16 (host transposes + casts).
  - K^T [256,S]: chunk 0 of head-pair 0 projected up front; remaining chunks
    just-in-time inside the (qc0, pr) kt-loops.
  - V [S,256]: projected just-in-time inside the (qc0, pr0) kt-loop.
  - scores^T [k,q] per (pair, ktile, qchunk): two row-packed (tile_position
    (0,0)/(64,0)) K=64 matmuls -> exp -> E^T bf16.  Most tiles exp on ACT
    (scale=1/8); tiles in OFFLOAD_KT use a clamped bf16 exp bit-trick on DVE
    (two tensor_scalar ops) to relieve the ACT bottleneck.
  - softmax denom: DVE folds E^T ktiles into ks (first fold sums tiles 0+1),
    then ones[128,128] matmul broadcasts the 128-partition reduction.
  - AV: col-packed (tile_position (0,0)/(0,64)) matmuls, V stationary,
    E^T streaming -> ctx^T accumulated over ktiles in PSUM.
  - ctx^T scaled by 1/denom (DVE, bf16 out).  O-proj of chunk qc is
    interleaved into chunk qc+1's pr0 kt-loop; Q-proj of qc+1 into qc's pr1
    kt-loop, so ACT never waits on projection phases.
"""

import sys

for _p in ("/opt/trn_rl_repo", "/root/.axon_site/_ro/trn_rl_repo"):
    if _p not in sys.path:
        sys.path.append(_p)

import math
from contextlib import ExitStack

import ml_dtypes
import numpy as np

import concourse.bass as bass
import concourse.mybir as mybir
import concourse.tile as tile
from concourse import bacc
from concourse.bass_utils import run_bass_kernel_spmd

BF16 = mybir.dt.bfloat16
F32 = mybir.dt.float32
I16 = mybir.dt.int16
AF = mybir.ActivationFunctionType
OP = mybir.AluOpType

B, S, D, H = 2, 2048, 1024, 16
DH = 64
HPC = 4  # heads per core
GD = HPC * DH  # 256 (head-group width)
NCORES = 8
NDT = D // 128  # 8 contraction tiles over D

# E^T ktiles whose exp runs on DVE (bit-trick) instead of ACT
OFFLOAD_KT = ()
# bf16 Schraudolph constants: i16 = min(max(s*EXA, EXLO), EXHI) + EXB;
# E = bitcast_bf16(i16) ~= exp(s/8), max rel err ~4%, zero-mean.
EXA = 0.125 * 128.0 / math.log(2.0)
EXB = 16256.0 - 486411.0 / 65536.0 + 0.5
EXLO = 128.0 - 16249.0
EXHI = 18304.0 - 16249.0


def _build(ctx: ExitStack, tc: "tile.TileContext", io: dict, s: int):
    nc = tc.nc
    GQ = min(512, s)
    NQC = s // GQ  # q chunks
    NKT = s // 128  # k tiles

    hsT, wq, wk, wv, wo = io["hsT"], io["wq"], io["wk"], io["wv"], io["wo"]
    bq, bk, bv, outT = io["bq"], io["bk"], io["bv"], io["outT"]

    consts = ctx.enter_context(tc.tile_pool(name="consts", bufs=1))
    sb = ctx.enter_context(tc.tile_pool(name="sb", bufs=1))
    etp = ctx.enter_context(tc.tile_pool(name="etp", bufs=6))
    ksp = ctx.enter_context(tc.tile_pool(name="ksp", bufs=2))
    outp = ctx.enter_context(tc.tile_pool(name="outp", bufs=4))
    # PSUM budget: 2x2 (scores, 2-bank tiles) + 2 (ctx) + 2 (proj/denom/
    # o-proj, shared tag) = 8 banks
    ps_mm = ctx.enter_context(tc.tile_pool(name="ps_mm", bufs=2, space="PSUM"))
    ps_ctx = ctx.enter_context(tc.tile_pool(name="ps_ctx", bufs=2, space="PSUM"))
    ps_o = ctx.enter_context(tc.tile_pool(name="ps_o", bufs=2, space="PSUM"))

    # ---- constants ----
    ones128 = consts.tile([128, 128], BF16, tag="ones128", name="ones128")
    nc.vector.memset(ones128[:], 1.0)

    bq_sb = consts.tile([128, 2], F32, tag="bq", name="bq")
    nc.sync.dma_start(bq_sb[:], bq.rearrange("(m p) -> p m", p=128))
    bk_sb = consts.tile([128, 2], F32, tag="bk", name="bk")
    nc.sync.dma_start(bk_sb[:], bk.rearrange("(m p) -> p m", p=128))
    zbias = consts.tile([128, 1], F32, tag="zbias", name="zbias")
    nc.vector.memset(zbias[:], 0.0)
    # bv arrives pre-broadcast [128, GD] from the host
    bv_bc = consts.tile([128, GD], F32, tag="bvbc", name="bvbc")
    nc.sync.dma_start(bv_bc[:], bv[:, :])

    # ---- load weights FIRST (small; K/Q-proj need them before attention
    # can start), then hs^T chunk by chunk ----
    wq_sb = [consts.tile([128, GD], BF16, tag=f"wq{d}", name=f"wq{d}") for d in range(NDT)]
    wk_sb = [consts.tile([128, GD], BF16, tag=f"wk{d}", name=f"wk{d}") for d in range(NDT)]
    wv_sb = [consts.tile([128, GD], BF16, tag=f"wv{d}", name=f"wv{d}") for d in range(NDT)]
    for d in range(NDT):
        rs = slice(d * 128, (d + 1) * 128)
        nc.sync.dma_start(wk_sb[d][:], wk[rs, :])
        nc.sync.dma_start(wq_sb[d][:], wq[rs, :])
        nc.sync.dma_start(wv_sb[d][:], wv[rs, :])
    wo_sb = [consts.tile([128, D], BF16, tag=f"wo{p}", name=f"wo{p}") for p in range(2)]
    for p in range(2):
        nc.sync.dma_start(wo_sb[p][:], wo[p * 128 : (p + 1) * 128, :])

    hsT_sb = [sb.tile([128, s], BF16, tag=f"hsT{d}", name=f"hsT{d}") for d in range(NDT)]
    for qc in range(NQC):
        cs = slice(qc * GQ, (qc + 1) * GQ)
        for d in range(NDT):
            nc.sync.dma_start(hsT_sb[d][:, cs], hsT[d * 128 : (d + 1) * 128, cs])

    qT_sb = [sb.tile([128, s], BF16, tag=f"qT{m}", name=f"qT{m}") for m in range(2)]
    kT_sb = [sb.tile([128, s], BF16, tag=f"kT{m}", name=f"kT{m}") for m in range(2)]
    v_sb = [sb.tile([128, GD], BF16, tag=f"v{st}", name=f"v{st}") for st in range(NKT)]

    def kproj(m, c):
        ms = slice(m * 128, (m + 1) * 128)
        cls = slice(c * GQ, (c + 1) * GQ)
        p = ps_o.tile([128, GQ], F32, tag="po", name="kp")
        for d in range(NDT):
            nc.tensor.matmul(
                p[:], lhsT=wk_sb[d][:, ms], rhs=hsT_sb[d][:, cls],
                start=(d == 0), stop=(d == NDT - 1),
            )
        nc.vector.tensor_scalar_add(kT_sb[m][:, cls], p[:], bk_sb[:, m : m + 1])

    def qproj(m, c):
        ms = slice(m * 128, (m + 1) * 128)
        cls = slice(c * GQ, (c + 1) * GQ)
        p = ps_o.tile([128, GQ], F32, tag="po", name="qp")
        for d in range(NDT):
            nc.tensor.matmul(
                p[:], lhsT=wq_sb[d][:, ms], rhs=hsT_sb[d][:, cls],
                start=(d == 0), stop=(d == NDT - 1),
            )
        nc.vector.tensor_scalar_add(qT_sb[m][:, cls], p[:], bq_sb[:, m : m + 1])

    def vproj(st):
        ss = slice(st * 128, (st + 1) * 128)
        vp = ps_o.tile([128, GD], F32, tag="po", name="vp")
        for d in range(NDT):
            nc.tensor.matmul(
                vp[:], lhsT=hsT_sb[d][:, ss], rhs=wv_sb[d][:],
                start=(d == 0), stop=(d == NDT - 1),
            )
        nc.vector.tensor_tensor(v_sb[st][:], vp[:], bv_bc[:], op=OP.add)

    ctx_sc_of = {}

    def oproj(qc, mt):
        cs = slice(qc * GQ, (qc + 1) * GQ)
        ms = slice(mt * 128, (mt + 1) * 128)
        o_ps = ps_o.tile([128, GQ], F32, tag="po", name="po")
        for pr in range(2):
            nc.tensor.matmul(
                o_ps[:], lhsT=wo_sb[pr][:, ms], rhs=ctx_sc_of[qc][pr][:],
                start=(pr == 0), stop=(pr == 1),
            )
        ost = outp.tile([128, GQ], BF16, tag="ost", name="ost")
        nc.vector.tensor_copy(ost[:], o_ps[:])
        nc.sync.dma_start(outT[ms, cs], ost[:])

    # serial head: K^T chunk 0 (pair 0) and Q^T chunk 0, so scores start ASAP
    kproj(0, 0)
    qproj(0, 0)
    qproj(1, 0)

    # interleave schedule: (qc, pr) -> {kt: [work closures]}
    def make_sched():
        sched = {(qc, pr): {} for qc in range(NQC) for pr in range(2)}

        def add(qc, pr, kt, fn):
            sched[(qc, pr)].setdefault(kt, []).append(fn)

        for st in range(NKT):  # V just-in-time in (qc0, pr0)
            add(0, 0, st, lambda st=st: vproj(st))
        for c in range(1, NQC):  # K chunks 1..3, pair 0
            add(0, 0, 4 * c - 3, lambda c=c: kproj(0, c))
        add(0, 0, NKT - 3, lambda: kproj(1, 0))  # K chunk 0, pair 1
        for c in range(1, NQC):  # K chunks 1..3, pair 1
            add(0, 1, 4 * c - 3, lambda c=c: kproj(1, c))
        for qc in range(NQC - 1):  # Q-proj of qc+1 late in (qc, pr1)
            add(qc, 1, NKT - 5, lambda qc=qc: qproj(0, qc + 1))
            add(qc, 1, NKT - 3, lambda qc=qc: qproj(1, qc + 1))
        for qc in range(1, NQC):  # O-proj of qc-1 split across (qc, pr0/pr1)
            for mt in range(D // 128):
                add(qc, mt // 4, mt % 4 + 1, lambda qc=qc, mt=mt: oproj(qc - 1, mt))
        return sched

    sched = make_sched()

    # ---- per q-chunk: attention (projections interleaved per sched) ----
    for qc in range(NQC):
        cs = slice(qc * GQ, (qc + 1) * GQ)
        ctx_sc = [sb.tile([128, GQ], BF16, tag=f"ctxs{p}_{qc % 2}", name=f"ctxs{p}_{qc % 2}") for p in range(2)]
        ctx_sc_of[qc] = ctx_sc
        for pr in range(2):
            # one PSUM bank per head: col-packed AV matmuls write disjoint
            # partition ranges, but each bank hosts a single accumulation group
            ctx_psA = ps_ctx.tile([128, GQ], F32, tag="ctx", name="ctxA")
            ctx_psB = ps_ctx.tile([128, GQ], F32, tag="ctx", name="ctxB")
            ks = ksp.tile([128, 2 * GQ], BF16, tag="ks", name="ks")
            # Software-pipelined over kt: the AV pair for kt-1 is emitted right
            # after the scores pair for kt; exp/fold of kt overlap the next
            # group's matmuls; interleaved projection work fills PE slack.
            ets = [None] * NKT
            for kt in range(NKT + 1):
                if kt < NKT:
                    ks_ = slice(kt * 128, (kt + 1) * 128)
                    # heads A|B share one 2-bank psum tile -> single wide exp
                    sp = ps_mm.tile([128, 2 * GQ], F32, tag="smm", name="smm")
                    nc.tensor.matmul(
                        sp[:, 0:GQ], lhsT=kT_sb[pr][0:64, ks_], rhs=qT_sb[pr][0:64, cs],
                        tile_position=(0, 0), start=True, stop=True,
                    )
                    nc.tensor.matmul(
                        sp[:, GQ : 2 * GQ], lhsT=kT_sb[pr][64:128, ks_], rhs=qT_sb[pr][64:128, cs],
                        tile_position=(64, 0), start=True, stop=True,
                    )
                    for fn in sched[(qc, pr)].get(kt, ()):
                        fn()
                if kt > 0:
                    pv = kt - 1
                    et = ets[pv]
                    nc.tensor.matmul(
                        ctx_psA[0:64, :], lhsT=v_sb[pv][:, pr * 128 : pr * 128 + 64],
                        rhs=et[:, 0:GQ], tile_position=(0, 0),
                        start=(pv == 0), stop=(pv == NKT - 1),
                    )
                    nc.tensor.matmul(
                        ctx_psB[64:128, :], lhsT=v_sb[pv][:, pr * 128 + 64 : pr * 128 + 128],
                        rhs=et[:, GQ : 2 * GQ], tile_position=(0, 64),
                        start=(pv == 0), stop=(pv == NKT - 1),
                    )
                if kt < NKT:
                    et = etp.tile([128, 2 * GQ], BF16, tag="et", name="et")
                    ets[kt] = et
                    if kt in OFFLOAD_KT:
                        # clamped bf16 exp bit-trick on DVE (2 tensor_scalar ops)
                        u = ksp.tile([128, 2 * GQ], F32, tag="shru", name="shru")
                        nc.vector.tensor_scalar(u[:], sp[:], EXA, EXLO, op0=OP.mult, op1=OP.max)
                        nc.vector.tensor_scalar(
                            et[:].bitcast(I16), u[:], EXHI, EXB, op0=OP.min, op1=OP.add
                        )
                    else:
                        # single wide exp over both heads (2-bank PSUM read)
                        nc.scalar.activation(et[:], sp[:], AF.Exp, bias=zbias[:, 0:1], scale=0.125)
                    if kt == 1:
                        nc.vector.tensor_tensor(ks[:], ets[0][:], ets[1][:], op=OP.add)
                    elif kt > 1:
                        nc.vector.tensor_tensor(ks[:], ks[:], et[:], op=OP.add)
            # softmax denominators (broadcast to all partitions) and ctx scaling
            rA = ksp.tile([128, GQ], F32, tag="rrA", name="rrA")
            rB = ksp.tile([128, GQ], F32, tag="rrB", name="rrB")
            for hh, r_sb in ((0, rA), (1, rB)):
                db_ps = ps_o.tile([128, GQ], F32, tag="po", name="po")
                nc.tensor.matmul(
                    db_ps[:], lhsT=ones128[:], rhs=ks[:, hh * GQ : (hh + 1) * GQ],
                    start=True, stop=True,
                )
                nc.vector.reciprocal_approx_fast(r_sb[:], db_ps[:])
            nc.vector.tensor_tensor(ctx_sc[pr][0:64, :], ctx_psA[0:64, :], rA[0:64, :], op=OP.mult)
            nc.vector.tensor_tensor(ctx_sc[pr][64:128, :], ctx_psB[64:128, :], rB[64:128, :], op=OP.mult)

    # tail: O-projection of the last chunk
    for mt in range(D // 128):
        oproj(NQC - 1, mt)


def build_gau_nc(s: int = S, debug: bool = False):
    nc = bacc.Bacc("TRN2", target_bir_lowering=False, debug=debug, num_devices=NCORES)
    io = {
        "hsT": nc.dram_tensor("hsT", [D, s], BF16, kind="ExternalInput").ap(),
        "wq": nc.dram_tensor("wq", [D, GD], BF16, kind="ExternalInput").ap(),
        "wk": nc.dram_tensor("wk", [D, GD], BF16, kind="ExternalInput").ap(),
        "wv": nc.dram_tensor("wv", [D, GD], BF16, kind="ExternalInput").ap(),
        "wo": nc.dram_tensor("wo", [GD, D], BF16, kind="ExternalInput").ap(),
        "bq": nc.dram_tensor("bq", [GD], F32, kind="ExternalInput").ap(),
        "bk": nc.dram_tensor("bk", [GD], F32, kind="ExternalInput").ap(),
        "bv": nc.dram_tensor("bv", [128, GD], F32, kind="ExternalInput").ap(),
        "outT": nc.dram_tensor("outT", [D, s], BF16, kind="ExternalOutput").ap(),
    }
    with tile.TileContext(nc) as tc:
        with ExitStack() as ctx:
            _build(ctx, tc, io, s)
    nc.compile()
    return nc


def make_in_maps(hidden_states, Wq, bq, Wk, bk, Wv, bv, Wo, gating_factor, gating_bias):
    """Shard full inputs into 8 per-core input maps (host-side prep)."""
    bf = ml_dtypes.bfloat16
    f32 = np.float32
    hs = np.asarray(hidden_states, f32)
    Wq, Wk, Wv, Wo = (np.asarray(a, f32) for a in (Wq, Wk, Wv, Wo))
    bq, bk, bv = (np.asarray(a, f32) for a in (bq, bk, bv))

    hsT_b = [np.ascontiguousarray(hs[b].T).astype(bf) for b in range(B)]
    in_maps = []
    for c in range(NCORES):
        b, g = divmod(c, NCORES // B)
        cols = slice(g * GD, (g + 1) * GD)
        in_maps.append(
            {
                "hsT": hsT_b[b],
                "wq": np.ascontiguousarray(Wq[:, cols]).astype(bf),
                "wk": np.ascontiguousarray(Wk[:, cols]).astype(bf),
                "wv": np.ascontiguousarray(Wv[:, cols]).astype(bf),
                "wo": np.ascontiguousarray(Wo[cols, :]).astype(bf),
                "bq": np.ascontiguousarray(bq[cols]),
                "bk": np.ascontiguousarray(bk[cols]),
                "bv": np.ascontiguousarray(np.broadcast_to(bv[cols], (128, GD))),
            }
        )
    return in_maps


_NC_CACHE: dict = {}


def _get_nc(s: int = S):
    if s not in _NC_CACHE:
        _NC_CACHE[s] = build_gau_nc(s)
    return _NC_CACHE[s]


def run_gau(in_maps, **kwargs):
    nc = _get_nc(S)
    return run_bass_kernel_spmd(nc, in_maps, core_ids=list(range(NCORES)), **kwargs)


def assemble_output(results, hidden_states, gating_factor, gating_bias, bo):
    """Sum per-batch head-group partials, transpose back, apply gating + bo.

    The per-(batch, query) gating scalar commutes through the AV matmul and
    the output projection, so it is applied here on the host.
    """
    bo = np.asarray(bo, np.float32)
    hs = np.asarray(hidden_states, np.float32)
    gf = np.float32(np.asarray(gating_factor, np.float32)[0])
    gb = np.float32(np.asarray(gating_bias, np.float32)[0])
    gate = 1.0 / (1.0 + np.exp(-(gf * hs.mean(axis=-1) + gb)))  # [B, S]
    gpb = NCORES // B
    out = np.empty((B, S, D), np.float32)
    for b in range(B):
        acc = results[gpb * b]["outT"].astype(np.float32)
        for g in range(1, gpb):
            acc = acc + results[gpb * b + g]["outT"].astype(np.float32)
        out[b] = acc.T * gate[b][:, None] + bo[None, :]
    return out


def kernel(hidden_states, Wq, bq, Wk, bk, Wv, bv, Wo, bo, gating_factor, gating_bias):
    in_maps = make_in_maps(
        hidden_states, Wq, bq, Wk, bk, Wv, bv, Wo, gating_factor, gating_bias
    )
    res = run_gau(in_maps)
    return assemble_output(res.results, hidden_states, gating_factor, gating_bias, bo)


# revision 17
# speedup vs baseline: 1.1899x; 1.0171x over previous
"""GAU attention (gated attention unit) Trainium2 Bass kernel.

Reference computation (B=2, S=2048, D=1024, H=16, DH=64):
    q = (hs @ Wq + bq), k = (hs @ Wk + bk), v = (hs @ Wv + bv)   per-head [B,S,H,DH]
    scores = q k^T / sqrt(DH);  probs = softmax(scores, axis=k)
    gating = sigmoid(gf * mean_d(hs) + gb)          # [B, S] per (batch, query)
    ctx = (probs * gating) @ v;  out = ctx @ Wo + bo

Sharding: 8 cores = 2 batches x 4 head-groups (4 heads each).  Each core
computes out^T partial [D, S] for its (batch, head-group); host sums the 4
partials per batch, applies the per-query gating scalar (gating commutes
through the linear AV + O-proj), and adds bo.

Per-core dataflow (matmuls bf16 with fp32 PSUM accumulation).  The kernel is
ACT(exp)-bound in steady state, so all projection work is interleaved into
the attention kt-loops to keep the exp pipeline dense:
  - hs^T [D,S] staged bf16 (host transposes + casts).
  - K^T [256,S]: chunk 0 of head-pair 0 projected up front; remaining chunks
    just-in-time inside the (qc0, pr) kt-loops.
  - V [S,256]: projected just-in-time inside the (qc0, pr0) kt-loop.
  - scores^T [k,q] per (pair, ktile, qchunk): two row-packed (tile_position
    (0,0)/(64,0)) K=64 matmuls -> exp -> E^T bf16.  Most tiles exp on ACT
    (scale=1/8); tiles in OFFLOAD_KT use a clamped bf16 exp bit-trick on DVE
    (two tensor_scalar ops) to relieve the ACT bottleneck.
  - softmax denom: DVE folds E^T ktiles into ks (first fold sums tiles 0+1),
    then ones[128,128] matmul broadcasts the 128-partition reduction.
  - AV: col-packed (tile_position (0,0)/(0,64)) matmuls, V stationary,
    E^T streaming -> ctx^T accumulated over ktiles in PSUM.
  - ctx^T scaled by 1/denom (DVE, bf16 out).  O-proj of chunk qc is
    interleaved into chunk qc+1's pr0 kt-loop; Q-proj of qc+1 into qc's pr1
    kt-loop, so ACT never waits on projection phases.
"""

import sys

for _p in ("/opt/trn_rl_repo", "/root/.axon_site/_ro/trn_rl_repo"):
    if _p not in sys.path:
        sys.path.append(_p)

import math
from contextlib import ExitStack

import ml_dtypes
import numpy as np

import concourse.bass as bass
import concourse.mybir as mybir
import concourse.tile as tile
from concourse import bacc
from concourse.bass_utils import run_bass_kernel_spmd

BF16 = mybir.dt.bfloat16
F32 = mybir.dt.float32
I16 = mybir.dt.int16
AF = mybir.ActivationFunctionType
OP = mybir.AluOpType

B, S, D, H = 2, 2048, 1024, 16
DH = 64
HPC = 4  # heads per core
GD = HPC * DH  # 256 (head-group width)
NCORES = 8
NDT = D // 128  # 8 contraction tiles over D

# E^T ktiles whose exp runs on DVE (bit-trick) instead of ACT
OFFLOAD_KT = ()
# bf16 Schraudolph constants: i16 = min(max(s*EXA, EXLO), EXHI) + EXB;
# E = bitcast_bf16(i16) ~= exp(s/8), max rel err ~4%, zero-mean.
EXA = 0.125 * 128.0 / math.log(2.0)
EXB = 16256.0 - 486411.0 / 65536.0 + 0.5
EXLO = 128.0 - 16249.0
EXHI = 18304.0 - 16249.0


def _build(ctx: ExitStack, tc: "tile.TileContext", io: dict, s: int):
    nc = tc.nc
    GQ = min(512, s)
    NQC = s // GQ  # q chunks
    NKT = s // 128  # k tiles

    hsT, wq, wk, wv, wo = io["hsT"], io["wq"], io["wk"], io["wv"], io["wo"]
    bq, bk, bv, outT = io["bq"], io["bk"], io["bv"], io["outT"]

    consts = ctx.enter_context(tc.tile_pool(name="consts", bufs=1))
    sb = ctx.enter_context(tc.tile_pool(name="sb", bufs=1))
    etp = ctx.enter_context(tc.tile_pool(name="etp", bufs=6))
    ksp = ctx.enter_context(tc.tile_pool(name="ksp", bufs=2))
    outp = ctx.enter_context(tc.tile_pool(name="outp", bufs=2))
    # PSUM budget: 2x2 (scores, 2-bank tiles) + 2 (ctx) + 2 (proj/denom/
    # o-proj, shared tag) = 8 banks
    ps_mm = ctx.enter_context(tc.tile_pool(name="ps_mm", bufs=2, space="PSUM"))
    ps_ctx = ctx.enter_context(tc.tile_pool(name="ps_ctx", bufs=2, space="PSUM"))
    ps_o = ctx.enter_context(tc.tile_pool(name="ps_o", bufs=2, space="PSUM"))

    # ---- constants ----
    ones128 = consts.tile([128, 128], BF16, tag="ones128", name="ones128")
    nc.vector.memset(ones128[:], 1.0)

    bq_sb = consts.tile([128, 2], F32, tag="bq", name="bq")
    nc.sync.dma_start(bq_sb[:], bq.rearrange("(m p) -> p m", p=128))
    bk_sb = consts.tile([128, 2], F32, tag="bk", name="bk")
    nc.sync.dma_start(bk_sb[:], bk.rearrange("(m p) -> p m", p=128))
    zbias = consts.tile([128, 1], F32, tag="zbias", name="zbias")
    nc.vector.memset(zbias[:], 0.0)
    # bv arrives pre-broadcast [128, GD] from the host
    bv_bc = consts.tile([128, GD], F32, tag="bvbc", name="bvbc")
    nc.sync.dma_start(bv_bc[:], bv[:, :])

    # ---- load weights FIRST (small; K/Q-proj need them before attention
    # can start), then hs^T chunk by chunk.  Each group is ONE dma_start:
    # per-dma sequencer issue costs ~1.2us serially, so batching the loads
    # moves the first score matmul ~25us earlier. ----
    wq_sb = consts.tile([128, NDT, GD], BF16, tag="wq", name="wq")
    wk_sb = consts.tile([128, NDT, GD], BF16, tag="wk", name="wk")
    wv_sb = consts.tile([128, NDT, GD], BF16, tag="wv", name="wv")
    nc.sync.dma_start(wk_sb[:], wk.rearrange("(d p) g -> p d g", p=128))
    nc.sync.dma_start(wq_sb[:], wq.rearrange("(d p) g -> p d g", p=128))
    nc.sync.dma_start(wv_sb[:], wv.rearrange("(d p) g -> p d g", p=128))
    wo_sb = consts.tile([128, 2, D], BF16, tag="wo", name="wo")
    nc.sync.dma_start(wo_sb[:], wo.rearrange("(t p) x -> p t x", p=128))

    hsT_sb = sb.tile([128, NDT, s], BF16, tag="hsT", name="hsT")
    for qc in range(NQC):
        cs = slice(qc * GQ, (qc + 1) * GQ)
        nc.sync.dma_start(
            hsT_sb[:, :, cs], hsT[:, cs].rearrange("(d p) s -> p d s", p=128)
        )

    qT_sb = [sb.tile([128, s], BF16, tag=f"qT{m}", name=f"qT{m}") for m in range(2)]
    kT_sb = [sb.tile([128, s], BF16, tag=f"kT{m}", name=f"kT{m}") for m in range(2)]
    v_sb = [sb.tile([128, GD], BF16, tag=f"v{st}", name=f"v{st}") for st in range(NKT)]

    def kproj(m, c):
        ms = slice(m * 128, (m + 1) * 128)
        cls = slice(c * GQ, (c + 1) * GQ)
        p = ps_o.tile([128, GQ], F32, tag="po", name="kp")
        for d in range(NDT):
            nc.tensor.matmul(
                p[:], lhsT=wk_sb[:, d, ms], rhs=hsT_sb[:, d, cls],
                start=(d == 0), stop=(d == NDT - 1),
            )
        nc.vector.tensor_scalar_add(kT_sb[m][:, cls], p[:], bk_sb[:, m : m + 1])

    def qproj(m, c):
        ms = slice(m * 128, (m + 1) * 128)
        cls = slice(c * GQ, (c + 1) * GQ)
        p = ps_o.tile([128, GQ], F32, tag="po", name="qp")
        for d in range(NDT):
            nc.tensor.matmul(
                p[:], lhsT=wq_sb[:, d, ms], rhs=hsT_sb[:, d, cls],
                start=(d == 0), stop=(d == NDT - 1),
            )
        nc.vector.tensor_scalar_add(qT_sb[m][:, cls], p[:], bq_sb[:, m : m + 1])

    def vproj(st):
        ss = slice(st * 128, (st + 1) * 128)
        vp = ps_o.tile([128, GD], F32, tag="po", name="vp")
        for d in range(NDT):
            nc.tensor.matmul(
                vp[:], lhsT=hsT_sb[:, d, ss], rhs=wv_sb[:, d, :],
                start=(d == 0), stop=(d == NDT - 1),
            )
        nc.vector.tensor_tensor(v_sb[st][:], vp[:], bv_bc[:], op=OP.add)

    ctx_sc_of = {}
    ost_of = {}

    def oproj(qc, mt):
        cs = slice(qc * GQ, (qc + 1) * GQ)
        ms = slice(mt * 128, (mt + 1) * 128)
        o_ps = ps_o.tile([128, GQ], F32, tag="po", name="po")
        for pr in range(2):
            nc.tensor.matmul(
                o_ps[:], lhsT=wo_sb[:, pr, ms], rhs=ctx_sc_of[qc][pr][:],
                start=(pr == 0), stop=(pr == 1),
            )
        if mt == 0:
            ost_of[qc] = outp.tile([128, D // 128, GQ], BF16, tag="ost", name="ost")
        ost = ost_of[qc]
        nc.vector.tensor_copy(ost[:, mt, :], o_ps[:])
        if mt == D // 128 - 1:
            # one batched out-DMA per q chunk (vs 8 small ones)
            nc.sync.dma_start(
                outT[:, cs].rearrange("(mt p) q -> p mt q", p=128), ost[:]
            )

    # serial head: K^T chunk 0 (pair 0) and Q^T chunk 0, so scores start ASAP
    kproj(0, 0)
    qproj(0, 0)
    qproj(1, 0)

    # interleave schedule: (qc, pr) -> {kt: [work closures]}
    def make_sched():
        sched = {(qc, pr): {} for qc in range(NQC) for pr in range(2)}

        def add(qc, pr, kt, fn):
            sched[(qc, pr)].setdefault(kt, []).append(fn)

        for st in range(NKT):  # V just-in-time in (qc0, pr0)
            add(0, 0, st, lambda st=st: vproj(st))
        for c in range(1, NQC):  # K chunks 1..3, pair 0
            add(0, 0, 4 * c - 3, lambda c=c: kproj(0, c))
        add(0, 0, NKT - 3, lambda: kproj(1, 0))  # K chunk 0, pair 1
        for c in range(1, NQC):  # K chunks 1..3, pair 1
            add(0, 1, 4 * c - 3, lambda c=c: kproj(1, c))
        for qc in range(NQC - 1):  # Q-proj of qc+1 late in (qc, pr1)
            add(qc, 1, NKT - 5, lambda qc=qc: qproj(0, qc + 1))
            add(qc, 1, NKT - 3, lambda qc=qc: qproj(1, qc + 1))
        for qc in range(1, NQC):  # O-proj of qc-1 split across (qc, pr0/pr1)
            for mt in range(D // 128):
                add(qc, mt // 4, mt % 4 + 1, lambda qc=qc, mt=mt: oproj(qc - 1, mt))
        return sched

    sched = make_sched()

    # ---- per q-chunk: attention (projections interleaved per sched) ----
    for qc in range(NQC):
        cs = slice(qc * GQ, (qc + 1) * GQ)
        ctx_sc = [sb.tile([128, GQ], BF16, tag=f"ctxs{p}_{qc % 2}", name=f"ctxs{p}_{qc % 2}") for p in range(2)]
        ctx_sc_of[qc] = ctx_sc
        for pr in range(2):
            # one PSUM bank per head: col-packed AV matmuls write disjoint
            # partition ranges, but each bank hosts a single accumulation group
            ctx_psA = ps_ctx.tile([128, GQ], F32, tag="ctx", name="ctxA")
            ctx_psB = ps_ctx.tile([128, GQ], F32, tag="ctx", name="ctxB")
            ks = ksp.tile([128, 2 * GQ], BF16, tag="ks", name="ks")
            # Software-pipelined over kt: the AV pair for kt-1 is emitted right
            # after the scores pair for kt; exp/fold of kt overlap the next
            # group's matmuls; interleaved projection work fills PE slack.
            ets = [None] * NKT
            for kt in range(NKT + 1):
                if kt < NKT:
                    ks_ = slice(kt * 128, (kt + 1) * 128)
                    # heads A|B share one 2-bank psum tile -> single wide exp
                    sp = ps_mm.tile([128, 2 * GQ], F32, tag="smm", name="smm")
                    nc.tensor.matmul(
                        sp[:, 0:GQ], lhsT=kT_sb[pr][0:64, ks_], rhs=qT_sb[pr][0:64, cs],
                        tile_position=(0, 0), start=True, stop=True,
                    )
                    nc.tensor.matmul(
                        sp[:, GQ : 2 * GQ], lhsT=kT_sb[pr][64:128, ks_], rhs=qT_sb[pr][64:128, cs],
                        tile_position=(64, 0), start=True, stop=True,
                    )
                    for fn in sched[(qc, pr)].get(kt, ()):
                        fn()
                if kt > 0:
                    pv = kt - 1
                    et = ets[pv]
                    nc.tensor.matmul(
                        ctx_psA[0:64, :], lhsT=v_sb[pv][:, pr * 128 : pr * 128 + 64],
                        rhs=et[:, 0:GQ], tile_position=(0, 0),
                        start=(pv == 0), stop=(pv == NKT - 1),
                    )
                    nc.tensor.matmul(
                        ctx_psB[64:128, :], lhsT=v_sb[pv][:, pr * 128 + 64 : pr * 128 + 128],
                        rhs=et[:, GQ : 2 * GQ], tile_position=(0, 64),
                        start=(pv == 0), stop=(pv == NKT - 1),
                    )
                if kt < NKT:
                    et = etp.tile([128, 2 * GQ], BF16, tag="et", name="et")
                    ets[kt] = et
                    if kt in OFFLOAD_KT:
                        # clamped bf16 exp bit-trick on DVE (2 tensor_scalar ops)
                        u = ksp.tile([128, 2 * GQ], F32, tag="shru", name="shru")
                        nc.vector.tensor_scalar(u[:], sp[:], EXA, EXLO, op0=OP.mult, op1=OP.max)
                        nc.vector.tensor_scalar(
                            et[:].bitcast(I16), u[:], EXHI, EXB, op0=OP.min, op1=OP.add
                        )
                    else:
                        # single wide exp over both heads (2-bank PSUM read)
                        nc.scalar.activation(et[:], sp[:], AF.Exp, bias=zbias[:, 0:1], scale=0.125)
                    if kt == 1:
                        nc.vector.tensor_tensor(ks[:], ets[0][:], ets[1][:], op=OP.add)
                    elif kt > 1:
                        nc.vector.tensor_tensor(ks[:], ks[:], et[:], op=OP.add)
            # softmax denominators (broadcast to all partitions) and ctx scaling
            rA = ksp.tile([128, GQ], F32, tag="rrA", name="rrA")
            rB = ksp.tile([128, GQ], F32, tag="rrB", name="rrB")
            for hh, r_sb in ((0, rA), (1, rB)):
                db_ps = ps_o.tile([128, GQ], F32, tag="po", name="po")
                nc.tensor.matmul(
                    db_ps[:], lhsT=ones128[:], rhs=ks[:, hh * GQ : (hh + 1) * GQ],
                    start=True, stop=True,
                )
                nc.vector.reciprocal_approx_fast(r_sb[:], db_ps[:])
            nc.vector.tensor_tensor(ctx_sc[pr][0:64, :], ctx_psA[0:64, :], rA[0:64, :], op=OP.mult)
            nc.vector.tensor_tensor(ctx_sc[pr][64:128, :], ctx_psB[64:128, :], rB[64:128, :], op=OP.mult)

    # tail: O-projection of the last chunk
    for mt in range(D // 128):
        oproj(NQC - 1, mt)


def build_gau_nc(s: int = S, debug: bool = False):
    nc = bacc.Bacc("TRN2", target_bir_lowering=False, debug=debug, num_devices=NCORES)
    io = {
        "hsT": nc.dram_tensor("hsT", [D, s], BF16, kind="ExternalInput").ap(),
        "wq": nc.dram_tensor("wq", [D, GD], BF16, kind="ExternalInput").ap(),
        "wk": nc.dram_tensor("wk", [D, GD], BF16, kind="ExternalInput").ap(),
        "wv": nc.dram_tensor("wv", [D, GD], BF16, kind="ExternalInput").ap(),
        "wo": nc.dram_tensor("wo", [GD, D], BF16, kind="ExternalInput").ap(),
        "bq": nc.dram_tensor("bq", [GD], F32, kind="ExternalInput").ap(),
        "bk": nc.dram_tensor("bk", [GD], F32, kind="ExternalInput").ap(),
        "bv": nc.dram_tensor("bv", [128, GD], F32, kind="ExternalInput").ap(),
        "outT": nc.dram_tensor("outT", [D, s], BF16, kind="ExternalOutput").ap(),
    }
    with tile.TileContext(nc) as tc:
        with ExitStack() as ctx:
            _build(ctx, tc, io, s)
    nc.compile()
    return nc


def make_in_maps(hidden_states, Wq, bq, Wk, bk, Wv, bv, Wo, gating_factor, gating_bias):
    """Shard full inputs into 8 per-core input maps (host-side prep)."""
    bf = ml_dtypes.bfloat16
    f32 = np.float32
    hs = np.asarray(hidden_states, f32)
    Wq, Wk, Wv, Wo = (np.asarray(a, f32) for a in (Wq, Wk, Wv, Wo))
    bq, bk, bv = (np.asarray(a, f32) for a in (bq, bk, bv))

    hsT_b = [np.ascontiguousarray(hs[b].T).astype(bf) for b in range(B)]
    in_maps = []
    for c in range(NCORES):
        b, g = divmod(c, NCORES // B)
        cols = slice(g * GD, (g + 1) * GD)
        in_maps.append(
            {
                "hsT": hsT_b[b],
                "wq": np.ascontiguousarray(Wq[:, cols]).astype(bf),
                "wk": np.ascontiguousarray(Wk[:, cols]).astype(bf),
                "wv": np.ascontiguousarray(Wv[:, cols]).astype(bf),
                "wo": np.ascontiguousarray(Wo[cols, :]).astype(bf),
                "bq": np.ascontiguousarray(bq[cols]),
                "bk": np.ascontiguousarray(bk[cols]),
                "bv": np.ascontiguousarray(np.broadcast_to(bv[cols], (128, GD))),
            }
        )
    return in_maps


_NC_CACHE: dict = {}


def _get_nc(s: int = S):
    if s not in _NC_CACHE:
        _NC_CACHE[s] = build_gau_nc(s)
    return _NC_CACHE[s]


def run_gau(in_maps, **kwargs):
    nc = _get_nc(S)
    return run_bass_kernel_spmd(nc, in_maps, core_ids=list(range(NCORES)), **kwargs)


def assemble_output(results, hidden_states, gating_factor, gating_bias, bo):
    """Sum per-batch head-group partials, transpose back, apply gating + bo.

    The per-(batch, query) gating scalar commutes through the AV matmul and
    the output projection, so it is applied here on the host.
    """
    bo = np.asarray(bo, np.float32)
    hs = np.asarray(hidden_states, np.float32)
    gf = np.float32(np.asarray(gating_factor, np.float32)[0])
    gb = np.float32(np.asarray(gating_bias, np.float32)[0])
    gate = 1.0 / (1.0 + np.exp(-(gf * hs.mean(axis=-1) + gb)))  # [B, S]
    gpb = NCORES // B
    out = np.empty((B, S, D), np.float32)
    for b in range(B):
        acc = results[gpb * b]["outT"].astype(np.float32)
        for g in range(1, gpb):
            acc = acc + results[gpb * b + g]["outT"].astype(np.float32)
        out[b] = acc.T * gate[b][:, None] + bo[None, :]
    return out


def kernel(hidden_states, Wq, bq, Wk, bk, Wv, bv, Wo, bo, gating_factor, gating_bias):
    in_maps = make_in_maps(
        hidden_states, Wq, bq, Wk, bk, Wv, bv, Wo, gating_factor, gating_bias
    )
    res = run_gau(in_maps)
    return assemble_output(res.results, hidden_states, gating_factor, gating_bias, bo)


# revision 31
# speedup vs baseline: 1.2288x; 1.0327x over previous
"""GAU attention (gated attention unit) Trainium2 Bass kernel.

Reference computation (B=2, S=2048, D=1024, H=16, DH=64):
    q = (hs @ Wq + bq), k = (hs @ Wk + bk), v = (hs @ Wv + bv)   per-head [B,S,H,DH]
    scores = q k^T / sqrt(DH);  probs = softmax(scores, axis=k)
    gating = sigmoid(gf * mean_d(hs) + gb)          # [B, S] per (batch, query)
    ctx = (probs * gating) @ v;  out = ctx @ Wo + bo

Sharding: 8 cores = 2 batches x 4 head-groups (4 heads each).  Each core
computes out^T partial [D, S] for its (batch, head-group); host sums the 4
partials per batch, applies the per-query gating scalar (gating commutes
through the linear AV + O-proj), and adds bo.

Per-core dataflow (matmuls bf16 with fp32 PSUM accumulation).  The kernel is
ACT(exp)-bound in steady state, so all projection work is interleaved into
the attention kt-loops to keep the exp pipeline dense:
  - hs^T [D,S] staged bf16 (host transposes + casts).
  - K^T [256,S]: chunk 0 of head-pair 0 projected up front; remaining chunks
    just-in-time inside the (qc0, pr) kt-loops.
  - V [S,256]: projected just-in-time inside the (qc0, pr0) kt-loop.
  - scores^T [k,q] per (pair, ktile, qchunk): two row-packed (tile_position
    (0,0)/(64,0)) K=64 matmuls -> exp -> E^T bf16.  Most tiles exp on ACT
    (scale=1/8); tiles in OFFLOAD_KT use a clamped bf16 exp bit-trick on DVE
    (two tensor_scalar ops) to relieve the ACT bottleneck.
  - softmax denom: DVE folds E^T ktiles into ks (first fold sums tiles 0+1),
    then ones[128,128] matmul broadcasts the 128-partition reduction.
  - AV: col-packed (tile_position (0,0)/(0,64)) matmuls, V stationary,
    E^T streaming -> ctx^T accumulated over ktiles in PSUM.
  - ctx^T scaled by 1/denom (DVE, bf16 out).  O-proj of chunk qc is
    interleaved into chunk qc+1's pr0 kt-loop; Q-proj of qc+1 into qc's pr1
    kt-loop, so ACT never waits on projection phases.
"""

import sys

for _p in ("/opt/trn_rl_repo", "/root/.axon_site/_ro/trn_rl_repo"):
    if _p not in sys.path:
        sys.path.append(_p)

import math
from contextlib import ExitStack

import ml_dtypes
import numpy as np

import concourse.bass as bass
import concourse.mybir as mybir
import concourse.tile as tile
from concourse import bacc
from concourse.bass_utils import run_bass_kernel_spmd

BF16 = mybir.dt.bfloat16
F32 = mybir.dt.float32
I16 = mybir.dt.int16
AF = mybir.ActivationFunctionType
OP = mybir.AluOpType

B, S, D, H = 2, 2048, 1024, 16
DH = 64
HPC = 4  # heads per core
GD = HPC * DH  # 256 (head-group width)
NCORES = 8
NDT = D // 128  # 8 contraction tiles over D

# E^T ktiles whose exp runs on DVE (bit-trick) instead of ACT
OFFLOAD_KT = ()
# bf16 Schraudolph constants: i16 = min(max(s*EXA, EXLO), EXHI) + EXB;
# E = bitcast_bf16(i16) ~= exp(s/8), max rel err ~4%, zero-mean.
EXA = 0.125 * 128.0 / math.log(2.0)
EXB = 16256.0 - 486411.0 / 65536.0 + 0.5
EXLO = 128.0 - 16249.0
EXHI = 18304.0 - 16249.0


def _build(ctx: ExitStack, tc: "tile.TileContext", io: dict, s: int):
    nc = tc.nc
    GQ = min(512, s)
    NQC = s // GQ  # q chunks
    NKT = s // 128  # k tiles

    hsT, wq, wk, wv, wo = io["hsT"], io["wq"], io["wk"], io["wv"], io["wo"]
    bq, bk, bv, outT = io["bq"], io["bk"], io["bv"], io["outT"]

    consts = ctx.enter_context(tc.tile_pool(name="consts", bufs=1))
    sb = ctx.enter_context(tc.tile_pool(name="sb", bufs=1))
    etp = ctx.enter_context(tc.tile_pool(name="etp", bufs=6))
    ksp = ctx.enter_context(tc.tile_pool(name="ksp", bufs=2))
    outp = ctx.enter_context(tc.tile_pool(name="outp", bufs=2))
    # PSUM budget: 2x2 (scores, 2-bank tiles) + 2 (ctx) + 2 (proj/denom/
    # o-proj, shared tag) = 8 banks
    ps_mm = ctx.enter_context(tc.tile_pool(name="ps_mm", bufs=2, space="PSUM"))
    ps_ctx = ctx.enter_context(tc.tile_pool(name="ps_ctx", bufs=2, space="PSUM"))
    ps_o = ctx.enter_context(tc.tile_pool(name="ps_o", bufs=2, space="PSUM"))

    # ---- constants ----
    ones128 = consts.tile([128, 128], BF16, tag="ones128", name="ones128")
    nc.vector.memset(ones128[:], 1.0)

    bq_sb = consts.tile([128, 2], F32, tag="bq", name="bq")
    bk_sb = consts.tile([128, 2], F32, tag="bk", name="bk")
    zbias = consts.tile([128, 1], F32, tag="zbias", name="zbias")
    nc.vector.memset(zbias[:], 0.0)
    bv_bc = consts.tile([128, GD], F32, tag="bvbc", name="bvbc")

    # ---- load weights FIRST (small; K/Q-proj need them before attention
    # can start), then hs^T chunk by chunk.  All tensors arrive host-permuted
    # to partition-major layouts so every DMA is one contiguous run per
    # partition (128 descriptors instead of ~1024): cuts both the serial
    # per-dma sequencer issue cost and descriptor-processing time. ----
    wq_sb = consts.tile([128, NDT, GD], BF16, tag="wq", name="wq")
    wk_sb = consts.tile([128, NDT, GD], BF16, tag="wk", name="wk")
    wv_sb = consts.tile([128, NDT, GD], BF16, tag="wv", name="wv")
    wo_sb = consts.tile([128, 2, D], BF16, tag="wo", name="wo")
    # hs^T chunk-major [p, chunk, dtile, col] so per-chunk DMAs are contiguous
    hsT_sb = sb.tile([128, NQC, NDT, GQ], BF16, tag="hsT", name="hsT")
    CW = NDT * GQ  # host-layout elements per chunk per partition

    nc.sync.dma_start(wk_sb[:], wk[:, :])
    nc.sync.dma_start(wq_sb[:], wq[:, :])
    nc.sync.dma_start(hsT_sb[:, 0, :, :], hsT[:, 0:CW])
    nc.sync.dma_start(wv_sb[:], wv[:, :])
    nc.sync.dma_start(wo_sb[:], wo[:, :])
    nc.sync.dma_start(bq_sb[:], bq.rearrange("(m p) -> p m", p=128))
    nc.sync.dma_start(bk_sb[:], bk.rearrange("(m p) -> p m", p=128))
    nc.sync.dma_start(bv_bc[:], bv[:, :])
    for qc in range(1, NQC):
        nc.sync.dma_start(hsT_sb[:, qc, :, :], hsT[:, qc * CW : (qc + 1) * CW])

    qT_sb = [sb.tile([128, s], BF16, tag=f"qT{m}", name=f"qT{m}") for m in range(2)]
    kT_sb = [sb.tile([128, s], BF16, tag=f"kT{m}", name=f"kT{m}") for m in range(2)]
    v_sb = [sb.tile([128, GD], BF16, tag=f"v{st}", name=f"v{st}") for st in range(NKT)]

    def kproj(m, c):
        ms = slice(m * 128, (m + 1) * 128)
        cls = slice(c * GQ, (c + 1) * GQ)
        p = ps_o.tile([128, GQ], F32, tag="po", name="kp")
        for d in range(NDT):
            nc.tensor.matmul(
                p[:], lhsT=wk_sb[:, d, ms], rhs=hsT_sb[:, c, d, :],
                start=(d == 0), stop=(d == NDT - 1),
            )
        nc.vector.tensor_scalar_add(kT_sb[m][:, cls], p[:], bk_sb[:, m : m + 1])

    def qproj(m, c):
        ms = slice(m * 128, (m + 1) * 128)
        cls = slice(c * GQ, (c + 1) * GQ)
        p = ps_o.tile([128, GQ], F32, tag="po", name="qp")
        for d in range(NDT):
            nc.tensor.matmul(
                p[:], lhsT=wq_sb[:, d, ms], rhs=hsT_sb[:, c, d, :],
                start=(d == 0), stop=(d == NDT - 1),
            )
        nc.vector.tensor_scalar_add(qT_sb[m][:, cls], p[:], bq_sb[:, m : m + 1])

    def vproj(st):
        sc_, so_ = st // 4, (st % 4) * 128
        vp = ps_o.tile([128, GD], F32, tag="po", name="vp")
        for d in range(NDT):
            nc.tensor.matmul(
                vp[:], lhsT=hsT_sb[:, sc_, d, so_ : so_ + 128], rhs=wv_sb[:, d, :],
                start=(d == 0), stop=(d == NDT - 1),
            )
        nc.vector.tensor_tensor(v_sb[st][:], vp[:], bv_bc[:], op=OP.add)

    ctx_sc_of = {}
    ost_of = {}
    OW = (D // 128) * GQ  # out elements per chunk per partition (host layout)

    def oproj(qc, mt):
        cs = slice(qc * GQ, (qc + 1) * GQ)
        ms = slice(mt * 128, (mt + 1) * 128)
        o_ps = ps_o.tile([128, GQ], F32, tag="po", name="po")
        for pr in range(2):
            nc.tensor.matmul(
                o_ps[:], lhsT=wo_sb[:, pr, ms], rhs=ctx_sc_of[qc][pr][:],
                start=(pr == 0), stop=(pr == 1),
            )
        if mt == 0:
            ost_of[qc] = outp.tile([128, D // 128, GQ], BF16, tag="ost", name="ost")
        ost = ost_of[qc]
        nc.vector.tensor_copy(ost[:, mt, :], o_ps[:])
        if mt == D // 128 - 1:
            # one batched out-DMA per q chunk, contiguous per partition
            nc.sync.dma_start(outT[:, qc * OW : (qc + 1) * OW], ost[:])

    # serial head: K^T chunk 0 (pair 0) and Q^T chunk 0, so scores start ASAP
    kproj(0, 0)
    qproj(0, 0)
    qproj(1, 0)

    # interleave schedule: (qc, pr) -> {kt: [work closures]}
    def make_sched():
        sched = {(qc, pr): {} for qc in range(NQC) for pr in range(2)}

        def add(qc, pr, kt, fn):
            sched[(qc, pr)].setdefault(kt, []).append(fn)

        for st in range(NKT):  # V just-in-time in (qc0, pr0)
            add(0, 0, st, lambda st=st: vproj(st))
        for c in range(1, NQC):  # K chunks 1..3, pair 0
            add(0, 0, 4 * c - 3, lambda c=c: kproj(0, c))
        add(0, 0, NKT - 3, lambda: kproj(1, 0))  # K chunk 0, pair 1
        for c in range(1, NQC):  # K chunks 1..3, pair 1
            add(0, 1, 4 * c - 3, lambda c=c: kproj(1, c))
        for qc in range(NQC - 1):  # Q-proj of qc+1 late in (qc, pr1)
            add(qc, 1, NKT - 5, lambda qc=qc: qproj(0, qc + 1))
            add(qc, 1, NKT - 3, lambda qc=qc: qproj(1, qc + 1))
        for qc in range(1, NQC):  # O-proj of qc-1 split across (qc, pr0/pr1)
            for mt in range(D // 128):
                add(qc, mt // 4, mt % 4 + 1, lambda qc=qc, mt=mt: oproj(qc - 1, mt))
        return sched

    sched = make_sched()

    # ---- per q-chunk: attention (projections interleaved per sched) ----
    for qc in range(NQC):
        cs = slice(qc * GQ, (qc + 1) * GQ)
        ctx_sc = [sb.tile([128, GQ], BF16, tag=f"ctxs{p}_{qc % 2}", name=f"ctxs{p}_{qc % 2}") for p in range(2)]
        ctx_sc_of[qc] = ctx_sc
        for pr in range(2):
            # one PSUM bank per head: col-packed AV matmuls write disjoint
            # partition ranges, but each bank hosts a single accumulation group
            ctx_psA = ps_ctx.tile([128, GQ], F32, tag="ctx", name="ctxA")
            ctx_psB = ps_ctx.tile([128, GQ], F32, tag="ctx", name="ctxB")
            ks = ksp.tile([128, 2 * GQ], BF16, tag="ks", name="ks")
            # Software-pipelined over kt: the AV pair for kt-1 is emitted right
            # after the scores pair for kt; exp/fold of kt overlap the next
            # group's matmuls; interleaved projection work fills PE slack.
            ets = [None] * NKT
            for kt in range(NKT + 1):
                if kt < NKT:
                    ks_ = slice(kt * 128, (kt + 1) * 128)
                    # heads A|B share one 2-bank psum tile -> single wide exp
                    sp = ps_mm.tile([128, 2 * GQ], F32, tag="smm", name="smm")
                    nc.tensor.matmul(
                        sp[:, 0:GQ], lhsT=kT_sb[pr][0:64, ks_], rhs=qT_sb[pr][0:64, cs],
                        tile_position=(0, 0), start=True, stop=True,
                    )
                    nc.tensor.matmul(
                        sp[:, GQ : 2 * GQ], lhsT=kT_sb[pr][64:128, ks_], rhs=qT_sb[pr][64:128, cs],
                        tile_position=(64, 0), start=True, stop=True,
                    )
                    for fn in sched[(qc, pr)].get(kt, ()):
                        fn()
                if kt > 0:
                    pv = kt - 1
                    et = ets[pv]
                    nc.tensor.matmul(
                        ctx_psA[0:64, :], lhsT=v_sb[pv][:, pr * 128 : pr * 128 + 64],
                        rhs=et[:, 0:GQ], tile_position=(0, 0),
                        start=(pv == 0), stop=(pv == NKT - 1),
                    )
                    nc.tensor.matmul(
                        ctx_psB[64:128, :], lhsT=v_sb[pv][:, pr * 128 + 64 : pr * 128 + 128],
                        rhs=et[:, GQ : 2 * GQ], tile_position=(0, 64),
                        start=(pv == 0), stop=(pv == NKT - 1),
                    )
                if kt < NKT:
                    et = etp.tile([128, 2 * GQ], BF16, tag="et", name="et")
                    ets[kt] = et
                    if kt in OFFLOAD_KT:
                        # clamped bf16 exp bit-trick on DVE (2 tensor_scalar ops)
                        u = ksp.tile([128, 2 * GQ], F32, tag="shru", name="shru")
                        nc.vector.tensor_scalar(u[:], sp[:], EXA, EXLO, op0=OP.mult, op1=OP.max)
                        nc.vector.tensor_scalar(
                            et[:].bitcast(I16), u[:], EXHI, EXB, op0=OP.min, op1=OP.add
                        )
                    else:
                        # single wide exp over both heads (2-bank PSUM read)
                        nc.scalar.activation(et[:], sp[:], AF.Exp, bias=zbias[:, 0:1], scale=0.125)
                    if kt == 1:
                        nc.vector.tensor_tensor(ks[:], ets[0][:], ets[1][:], op=OP.add)
                    elif kt > 1:
                        nc.vector.tensor_tensor(ks[:], ks[:], et[:], op=OP.add)
            # softmax denominators (broadcast to all partitions) and ctx scaling
            rA = ksp.tile([128, GQ], F32, tag="rrA", name="rrA")
            rB = ksp.tile([128, GQ], F32, tag="rrB", name="rrB")
            for hh, r_sb in ((0, rA), (1, rB)):
                db_ps = ps_o.tile([128, GQ], F32, tag="po", name="po")
                nc.tensor.matmul(
                    db_ps[:], lhsT=ones128[:], rhs=ks[:, hh * GQ : (hh + 1) * GQ],
                    start=True, stop=True,
                )
                nc.vector.reciprocal_approx_fast(r_sb[:], db_ps[:])
            nc.vector.tensor_tensor(ctx_sc[pr][0:64, :], ctx_psA[0:64, :], rA[0:64, :], op=OP.mult)
            nc.vector.tensor_tensor(ctx_sc[pr][64:128, :], ctx_psB[64:128, :], rB[64:128, :], op=OP.mult)

    # tail: O-projection of the last chunk
    for mt in range(D // 128):
        oproj(NQC - 1, mt)


def build_gau_nc(s: int = S, debug: bool = False):
    nc = bacc.Bacc("TRN2", target_bir_lowering=False, debug=debug, num_devices=NCORES)
    io = {
        "hsT": nc.dram_tensor("hsT", [128, NDT * s], BF16, kind="ExternalInput").ap(),
        "wq": nc.dram_tensor("wq", [128, NDT * GD], BF16, kind="ExternalInput").ap(),
        "wk": nc.dram_tensor("wk", [128, NDT * GD], BF16, kind="ExternalInput").ap(),
        "wv": nc.dram_tensor("wv", [128, NDT * GD], BF16, kind="ExternalInput").ap(),
        "wo": nc.dram_tensor("wo", [128, 2 * D], BF16, kind="ExternalInput").ap(),
        "bq": nc.dram_tensor("bq", [GD], F32, kind="ExternalInput").ap(),
        "bk": nc.dram_tensor("bk", [GD], F32, kind="ExternalInput").ap(),
        "bv": nc.dram_tensor("bv", [128, GD], F32, kind="ExternalInput").ap(),
        "outT": nc.dram_tensor("outT", [128, (D // 128) * s], BF16, kind="ExternalOutput").ap(),
    }
    with tile.TileContext(nc) as tc:
        with ExitStack() as ctx:
            _build(ctx, tc, io, s)
    nc.compile()
    return nc


def make_in_maps(hidden_states, Wq, bq, Wk, bk, Wv, bv, Wo, gating_factor, gating_bias):
    """Shard full inputs into 8 per-core input maps (host-side prep)."""
    bf = ml_dtypes.bfloat16
    f32 = np.float32
    hs = np.asarray(hidden_states, f32)
    Wq, Wk, Wv, Wo = (np.asarray(a, f32) for a in (Wq, Wk, Wv, Wo))
    bq, bk, bv = (np.asarray(a, f32) for a in (bq, bk, bv))

    NQC, GQ = S // 512, 512

    def perm_w(a, nblk):  # (nblk*128, X) -> [128, nblk*X] partition-major
        X = a.shape[1]
        return np.ascontiguousarray(
            a.reshape(nblk, 128, X).transpose(1, 0, 2).reshape(128, nblk * X)
        ).astype(bf)

    def perm_hsT(hT):  # [D, S] -> [128, NQC*NDT*GQ] chunk-major
        t = hT.reshape(NDT, 128, NQC, GQ).transpose(1, 2, 0, 3)
        return np.ascontiguousarray(t.reshape(128, NQC * NDT * GQ)).astype(bf)

    hsT_b = [perm_hsT(hs[b].T) for b in range(B)]
    in_maps = []
    for c in range(NCORES):
        b, g = divmod(c, NCORES // B)
        cols = slice(g * GD, (g + 1) * GD)
        in_maps.append(
            {
                "hsT": hsT_b[b],
                "wq": perm_w(np.ascontiguousarray(Wq[:, cols]), NDT),
                "wk": perm_w(np.ascontiguousarray(Wk[:, cols]), NDT),
                "wv": perm_w(np.ascontiguousarray(Wv[:, cols]), NDT),
                "wo": perm_w(np.ascontiguousarray(Wo[cols, :]), 2),
                "bq": np.ascontiguousarray(bq[cols]),
                "bk": np.ascontiguousarray(bk[cols]),
                "bv": np.ascontiguousarray(np.broadcast_to(bv[cols], (128, GD))),
            }
        )
    return in_maps


_NC_CACHE: dict = {}


def _get_nc(s: int = S):
    if s not in _NC_CACHE:
        _NC_CACHE[s] = build_gau_nc(s)
    return _NC_CACHE[s]


def run_gau(in_maps, **kwargs):
    nc = _get_nc(S)
    return run_bass_kernel_spmd(nc, in_maps, core_ids=list(range(NCORES)), **kwargs)


def assemble_output(results, hidden_states, gating_factor, gating_bias, bo):
    """Sum per-batch head-group partials, transpose back, apply gating + bo.

    The per-(batch, query) gating scalar commutes through the AV matmul and
    the output projection, so it is applied here on the host.
    """
    bo = np.asarray(bo, np.float32)
    hs = np.asarray(hidden_states, np.float32)
    gf = np.float32(np.asarray(gating_factor, np.float32)[0])
    gb = np.float32(np.asarray(gating_bias, np.float32)[0])
    gate = 1.0 / (1.0 + np.exp(-(gf * hs.mean(axis=-1) + gb)))  # [B, S]
    gpb = NCORES // B
    out = np.empty((B, S, D), np.float32)
    MT, NQC, GQ = D // 128, S // 512, 512
    for b in range(B):
        acc = results[gpb * b]["outT"].astype(np.float32)
        for g in range(1, gpb):
            acc = acc + results[gpb * b + g]["outT"].astype(np.float32)
        # [128, NQC, MT, GQ] -> out^T [D, S]
        accT = acc.reshape(128, NQC, MT, GQ).transpose(2, 0, 1, 3).reshape(D, S)
        out[b] = accT.T * gate[b][:, None] + bo[None, :]
    return out


def kernel(hidden_states, Wq, bq, Wk, bk, Wv, bv, Wo, bo, gating_factor, gating_bias):
    in_maps = make_in_maps(
        hidden_states, Wq, bq, Wk, bk, Wv, bv, Wo, gating_factor, gating_bias
    )
    res = run_gau(in_maps)
    return assemble_output(res.results, hidden_states, gating_factor, gating_bias, bo)


# revision 33
# speedup vs baseline: 1.2410x; 1.0099x over previous
"""GAU attention (gated attention unit) Trainium2 Bass kernel.

Reference computation (B=2, S=2048, D=1024, H=16, DH=64):
    q = (hs @ Wq + bq), k = (hs @ Wk + bk), v = (hs @ Wv + bv)   per-head [B,S,H,DH]
    scores = q k^T / sqrt(DH);  probs = softmax(scores, axis=k)
    gating = sigmoid(gf * mean_d(hs) + gb)          # [B, S] per (batch, query)
    ctx = (probs * gating) @ v;  out = ctx @ Wo + bo

Sharding: 8 cores = 2 batches x 4 head-groups (4 heads each).  Each core
computes out^T partial [D, S] for its (batch, head-group); host sums the 4
partials per batch, applies the per-query gating scalar (gating commutes
through the linear AV + O-proj), and adds bo.

Per-core dataflow (matmuls bf16 with fp32 PSUM accumulation).  The kernel is
ACT(exp)-bound in steady state, so all projection work is interleaved into
the attention kt-loops to keep the exp pipeline dense:
  - hs^T [D,S] staged bf16 (host transposes + casts).
  - K^T [256,S]: chunk 0 of head-pair 0 projected up front; remaining chunks
    just-in-time inside the (qc0, pr) kt-loops.
  - V [S,256]: projected just-in-time inside the (qc0, pr0) kt-loop.
  - scores^T [k,q] per (pair, ktile, qchunk): two row-packed (tile_position
    (0,0)/(64,0)) K=64 matmuls -> exp -> E^T bf16.  Most tiles exp on ACT
    (scale=1/8); tiles in OFFLOAD_KT use a clamped bf16 exp bit-trick on DVE
    (two tensor_scalar ops) to relieve the ACT bottleneck.
  - softmax denom: DVE folds E^T ktiles into ks (first fold sums tiles 0+1),
    then ones[128,128] matmul broadcasts the 128-partition reduction.
  - AV: col-packed (tile_position (0,0)/(0,64)) matmuls, V stationary,
    E^T streaming -> ctx^T accumulated over ktiles in PSUM.
  - ctx^T scaled by 1/denom (DVE, bf16 out).  O-proj of chunk qc is
    interleaved into chunk qc+1's pr0 kt-loop; Q-proj of qc+1 into qc's pr1
    kt-loop, so ACT never waits on projection phases.
"""

import sys

for _p in ("/opt/trn_rl_repo", "/root/.axon_site/_ro/trn_rl_repo"):
    if _p not in sys.path:
        sys.path.append(_p)

import math
from contextlib import ExitStack

import ml_dtypes
import numpy as np

import concourse.bass as bass
import concourse.mybir as mybir
import concourse.tile as tile
from concourse import bacc
from concourse.bass_utils import run_bass_kernel_spmd

BF16 = mybir.dt.bfloat16
F32 = mybir.dt.float32
I16 = mybir.dt.int16
AF = mybir.ActivationFunctionType
OP = mybir.AluOpType

B, S, D, H = 2, 2048, 1024, 16
DH = 64
HPC = 4  # heads per core
GD = HPC * DH  # 256 (head-group width)
NCORES = 8
NDT = D // 128  # 8 contraction tiles over D

# E^T ktiles whose exp runs on DVE (bit-trick) instead of ACT
OFFLOAD_KT = ()
# bf16 Schraudolph constants: i16 = min(max(s*EXA, EXLO), EXHI) + EXB;
# E = bitcast_bf16(i16) ~= exp(s/8), max rel err ~4%, zero-mean.
EXA = 0.125 * 128.0 / math.log(2.0)
EXB = 16256.0 - 486411.0 / 65536.0 + 0.5
EXLO = 128.0 - 16249.0
EXHI = 18304.0 - 16249.0


def _build(ctx: ExitStack, tc: "tile.TileContext", io: dict, s: int):
    nc = tc.nc
    GQ = min(512, s)
    NQC = s // GQ  # q chunks
    NKT = s // 128  # k tiles

    hsT, wq, wk, wv, wo = io["hsT"], io["wq"], io["wk"], io["wv"], io["wo"]
    bq, bk, bv, outT = io["bq"], io["bk"], io["bv"], io["outT"]

    consts = ctx.enter_context(tc.tile_pool(name="consts", bufs=1))
    sb = ctx.enter_context(tc.tile_pool(name="sb", bufs=1))
    etp = ctx.enter_context(tc.tile_pool(name="etp", bufs=6))
    ksp = ctx.enter_context(tc.tile_pool(name="ksp", bufs=2))
    outp = ctx.enter_context(tc.tile_pool(name="outp", bufs=2))
    # PSUM budget: 2x2 (scores, 2-bank tiles) + 2 (ctx) + 2 (proj/denom/
    # o-proj, shared tag) = 8 banks
    ps_mm = ctx.enter_context(tc.tile_pool(name="ps_mm", bufs=2, space="PSUM"))
    ps_ctx = ctx.enter_context(tc.tile_pool(name="ps_ctx", bufs=2, space="PSUM"))
    ps_o = ctx.enter_context(tc.tile_pool(name="ps_o", bufs=2, space="PSUM"))

    # ---- constants ----
    ones128 = consts.tile([128, 128], BF16, tag="ones128", name="ones128")
    nc.vector.memset(ones128[:], 1.0)

    bq_sb = consts.tile([128, 2], F32, tag="bq", name="bq")
    bk_sb = consts.tile([128, 2], F32, tag="bk", name="bk")
    zbias = consts.tile([128, 1], F32, tag="zbias", name="zbias")
    nc.vector.memset(zbias[:], 0.0)
    bv_bc = consts.tile([128, GD], F32, tag="bvbc", name="bvbc")

    # ---- load weights FIRST (small; K/Q-proj need them before attention
    # can start), then hs^T chunk by chunk.  All tensors arrive host-permuted
    # to partition-major layouts so every DMA is one contiguous run per
    # partition (128 descriptors instead of ~1024): cuts both the serial
    # per-dma sequencer issue cost and descriptor-processing time. ----
    wq_sb = consts.tile([128, NDT, GD], BF16, tag="wq", name="wq")
    wk_sb = consts.tile([128, NDT, GD], BF16, tag="wk", name="wk")
    wv_sb = consts.tile([128, NDT, GD], BF16, tag="wv", name="wv")
    wo_sb = consts.tile([128, 2, D], BF16, tag="wo", name="wo")
    # hs^T chunk-major [p, chunk, dtile, col] so per-chunk DMAs are contiguous
    hsT_sb = sb.tile([128, NQC, NDT, GQ], BF16, tag="hsT", name="hsT")
    CW = NDT * GQ  # host-layout elements per chunk per partition

    # PE warmup: throwaway matmuls during the input-DMA wait keep the PE
    # busy so the HAM clock gate releases (1.2 -> 2.4 GHz) before the first
    # real projection instead of ~3.4us into it.
    warm = ps_o.tile([128, GQ], F32, tag="po", name="warm")
    for _ in range(56):
        nc.tensor.matmul(warm[:, 0:128], lhsT=ones128[:], rhs=ones128[:], start=True, stop=True)

    nc.sync.dma_start(wk_sb[:], wk[:, :])
    nc.sync.dma_start(wq_sb[:], wq[:, :])
    nc.sync.dma_start(hsT_sb[:, 0, :, :], hsT[:, 0:CW])
    nc.sync.dma_start(wv_sb[:], wv[:, :])
    nc.sync.dma_start(wo_sb[:], wo[:, :])
    nc.sync.dma_start(bq_sb[:], bq.rearrange("(m p) -> p m", p=128))
    nc.sync.dma_start(bk_sb[:], bk.rearrange("(m p) -> p m", p=128))
    nc.sync.dma_start(bv_bc[:], bv[:, :])
    for qc in range(1, NQC):
        nc.sync.dma_start(hsT_sb[:, qc, :, :], hsT[:, qc * CW : (qc + 1) * CW])

    qT_sb = [sb.tile([128, s], BF16, tag=f"qT{m}", name=f"qT{m}") for m in range(2)]
    kT_sb = [sb.tile([128, s], BF16, tag=f"kT{m}", name=f"kT{m}") for m in range(2)]
    v_sb = [sb.tile([128, GD], BF16, tag=f"v{st}", name=f"v{st}") for st in range(NKT)]

    def kproj(m, c):
        ms = slice(m * 128, (m + 1) * 128)
        cls = slice(c * GQ, (c + 1) * GQ)
        p = ps_o.tile([128, GQ], F32, tag="po", name="kp")
        for d in range(NDT):
            nc.tensor.matmul(
                p[:], lhsT=wk_sb[:, d, ms], rhs=hsT_sb[:, c, d, :],
                start=(d == 0), stop=(d == NDT - 1),
            )
        nc.vector.tensor_scalar_add(kT_sb[m][:, cls], p[:], bk_sb[:, m : m + 1])

    def qproj(m, c):
        ms = slice(m * 128, (m + 1) * 128)
        cls = slice(c * GQ, (c + 1) * GQ)
        p = ps_o.tile([128, GQ], F32, tag="po", name="qp")
        for d in range(NDT):
            nc.tensor.matmul(
                p[:], lhsT=wq_sb[:, d, ms], rhs=hsT_sb[:, c, d, :],
                start=(d == 0), stop=(d == NDT - 1),
            )
        nc.vector.tensor_scalar_add(qT_sb[m][:, cls], p[:], bq_sb[:, m : m + 1])

    def vproj(st):
        sc_, so_ = st // 4, (st % 4) * 128
        vp = ps_o.tile([128, GD], F32, tag="po", name="vp")
        for d in range(NDT):
            nc.tensor.matmul(
                vp[:], lhsT=hsT_sb[:, sc_, d, so_ : so_ + 128], rhs=wv_sb[:, d, :],
                start=(d == 0), stop=(d == NDT - 1),
            )
        nc.vector.tensor_tensor(v_sb[st][:], vp[:], bv_bc[:], op=OP.add)

    ctx_sc_of = {}
    ost_of = {}
    OW = (D // 128) * GQ  # out elements per chunk per partition (host layout)

    def oproj(qc, mt):
        cs = slice(qc * GQ, (qc + 1) * GQ)
        ms = slice(mt * 128, (mt + 1) * 128)
        o_ps = ps_o.tile([128, GQ], F32, tag="po", name="po")
        for pr in range(2):
            nc.tensor.matmul(
                o_ps[:], lhsT=wo_sb[:, pr, ms], rhs=ctx_sc_of[qc][pr][:],
                start=(pr == 0), stop=(pr == 1),
            )
        if mt == 0:
            ost_of[qc] = outp.tile([128, D // 128, GQ], BF16, tag="ost", name="ost")
        ost = ost_of[qc]
        nc.vector.tensor_copy(ost[:, mt, :], o_ps[:])
        # two out-DMAs per q chunk so the second half's copies overlap the
        # first half's transfer (shrinks the kernel tail)
        half = D // 256
        if mt == half - 1:
            nc.sync.dma_start(outT[:, qc * OW : qc * OW + OW // 2], ost[:, 0:half, :])
        elif mt == D // 128 - 1:
            nc.sync.dma_start(outT[:, qc * OW + OW // 2 : (qc + 1) * OW], ost[:, half:, :])

    # serial head: K^T chunk 0 (pair 0) and Q^T chunk 0, so scores start ASAP
    kproj(0, 0)
    qproj(0, 0)
    qproj(1, 0)

    # interleave schedule: (qc, pr) -> {kt: [work closures]}
    def make_sched():
        sched = {(qc, pr): {} for qc in range(NQC) for pr in range(2)}

        def add(qc, pr, kt, fn):
            sched[(qc, pr)].setdefault(kt, []).append(fn)

        for st in range(NKT):  # V just-in-time in (qc0, pr0)
            add(0, 0, st, lambda st=st: vproj(st))
        for c in range(1, NQC):  # K chunks 1..3, pair 0
            add(0, 0, 4 * c - 3, lambda c=c: kproj(0, c))
        add(0, 0, NKT - 3, lambda: kproj(1, 0))  # K chunk 0, pair 1
        for c in range(1, NQC):  # K chunks 1..3, pair 1
            add(0, 1, 4 * c - 3, lambda c=c: kproj(1, c))
        for qc in range(NQC - 1):  # Q-proj of qc+1 late in (qc, pr1)
            add(qc, 1, NKT - 5, lambda qc=qc: qproj(0, qc + 1))
            add(qc, 1, NKT - 3, lambda qc=qc: qproj(1, qc + 1))
        for qc in range(1, NQC):  # O-proj of qc-1 split across (qc, pr0/pr1)
            for mt in range(D // 128):
                add(qc, mt // 4, mt % 4 + 1, lambda qc=qc, mt=mt: oproj(qc - 1, mt))
        return sched

    sched = make_sched()

    # ---- per q-chunk: attention (projections interleaved per sched) ----
    for qc in range(NQC):
        cs = slice(qc * GQ, (qc + 1) * GQ)
        ctx_sc = [sb.tile([128, GQ], BF16, tag=f"ctxs{p}_{qc % 2}", name=f"ctxs{p}_{qc % 2}") for p in range(2)]
        ctx_sc_of[qc] = ctx_sc
        for pr in range(2):
            # one PSUM bank per head: col-packed AV matmuls write disjoint
            # partition ranges, but each bank hosts a single accumulation group
            ctx_psA = ps_ctx.tile([128, GQ], F32, tag="ctx", name="ctxA")
            ctx_psB = ps_ctx.tile([128, GQ], F32, tag="ctx", name="ctxB")
            ks = ksp.tile([128, 2 * GQ], BF16, tag="ks", name="ks")
            # Software-pipelined over kt: the AV pair for kt-1 is emitted right
            # after the scores pair for kt; exp/fold of kt overlap the next
            # group's matmuls; interleaved projection work fills PE slack.
            ets = [None] * NKT
            for kt in range(NKT + 1):
                if kt < NKT:
                    ks_ = slice(kt * 128, (kt + 1) * 128)
                    # heads A|B share one 2-bank psum tile -> single wide exp
                    sp = ps_mm.tile([128, 2 * GQ], F32, tag="smm", name="smm")
                    nc.tensor.matmul(
                        sp[:, 0:GQ], lhsT=kT_sb[pr][0:64, ks_], rhs=qT_sb[pr][0:64, cs],
                        tile_position=(0, 0), start=True, stop=True,
                    )
                    nc.tensor.matmul(
                        sp[:, GQ : 2 * GQ], lhsT=kT_sb[pr][64:128, ks_], rhs=qT_sb[pr][64:128, cs],
                        tile_position=(64, 0), start=True, stop=True,
                    )
                    for fn in sched[(qc, pr)].get(kt, ()):
                        fn()
                if kt > 0:
                    pv = kt - 1
                    et = ets[pv]
                    nc.tensor.matmul(
                        ctx_psA[0:64, :], lhsT=v_sb[pv][:, pr * 128 : pr * 128 + 64],
                        rhs=et[:, 0:GQ], tile_position=(0, 0),
                        start=(pv == 0), stop=(pv == NKT - 1),
                    )
                    nc.tensor.matmul(
                        ctx_psB[64:128, :], lhsT=v_sb[pv][:, pr * 128 + 64 : pr * 128 + 128],
                        rhs=et[:, GQ : 2 * GQ], tile_position=(0, 64),
                        start=(pv == 0), stop=(pv == NKT - 1),
                    )
                if kt < NKT:
                    et = etp.tile([128, 2 * GQ], BF16, tag="et", name="et")
                    ets[kt] = et
                    if kt in OFFLOAD_KT:
                        # clamped bf16 exp bit-trick on DVE (2 tensor_scalar ops)
                        u = ksp.tile([128, 2 * GQ], F32, tag="shru", name="shru")
                        nc.vector.tensor_scalar(u[:], sp[:], EXA, EXLO, op0=OP.mult, op1=OP.max)
                        nc.vector.tensor_scalar(
                            et[:].bitcast(I16), u[:], EXHI, EXB, op0=OP.min, op1=OP.add
                        )
                    else:
                        # single wide exp over both heads (2-bank PSUM read)
                        nc.scalar.activation(et[:], sp[:], AF.Exp, bias=zbias[:, 0:1], scale=0.125)
                    if kt == 1:
                        nc.vector.tensor_tensor(ks[:], ets[0][:], ets[1][:], op=OP.add)
                    elif kt > 1:
                        nc.vector.tensor_tensor(ks[:], ks[:], et[:], op=OP.add)
            # softmax denominators (broadcast to all partitions) and ctx scaling
            rA = ksp.tile([128, GQ], F32, tag="rrA", name="rrA")
            rB = ksp.tile([128, GQ], F32, tag="rrB", name="rrB")
            for hh, r_sb in ((0, rA), (1, rB)):
                db_ps = ps_o.tile([128, GQ], F32, tag="po", name="po")
                nc.tensor.matmul(
                    db_ps[:], lhsT=ones128[:], rhs=ks[:, hh * GQ : (hh + 1) * GQ],
                    start=True, stop=True,
                )
                nc.vector.reciprocal_approx_fast(r_sb[:], db_ps[:])
            nc.vector.tensor_tensor(ctx_sc[pr][0:64, :], ctx_psA[0:64, :], rA[0:64, :], op=OP.mult)
            nc.vector.tensor_tensor(ctx_sc[pr][64:128, :], ctx_psB[64:128, :], rB[64:128, :], op=OP.mult)

    # tail: O-projection of the last chunk
    for mt in range(D // 128):
        oproj(NQC - 1, mt)


def build_gau_nc(s: int = S, debug: bool = False):
    nc = bacc.Bacc("TRN2", target_bir_lowering=False, debug=debug, num_devices=NCORES)
    io = {
        "hsT": nc.dram_tensor("hsT", [128, NDT * s], BF16, kind="ExternalInput").ap(),
        "wq": nc.dram_tensor("wq", [128, NDT * GD], BF16, kind="ExternalInput").ap(),
        "wk": nc.dram_tensor("wk", [128, NDT * GD], BF16, kind="ExternalInput").ap(),
        "wv": nc.dram_tensor("wv", [128, NDT * GD], BF16, kind="ExternalInput").ap(),
        "wo": nc.dram_tensor("wo", [128, 2 * D], BF16, kind="ExternalInput").ap(),
        "bq": nc.dram_tensor("bq", [GD], F32, kind="ExternalInput").ap(),
        "bk": nc.dram_tensor("bk", [GD], F32, kind="ExternalInput").ap(),
        "bv": nc.dram_tensor("bv", [128, GD], F32, kind="ExternalInput").ap(),
        "outT": nc.dram_tensor("outT", [128, (D // 128) * s], BF16, kind="ExternalOutput").ap(),
    }
    with tile.TileContext(nc) as tc:
        with ExitStack() as ctx:
            _build(ctx, tc, io, s)
    nc.compile()
    return nc


def make_in_maps(hidden_states, Wq, bq, Wk, bk, Wv, bv, Wo, gating_factor, gating_bias):
    """Shard full inputs into 8 per-core input maps (host-side prep)."""
    bf = ml_dtypes.bfloat16
    f32 = np.float32
    hs = np.asarray(hidden_states, f32)
    Wq, Wk, Wv, Wo = (np.asarray(a, f32) for a in (Wq, Wk, Wv, Wo))
    bq, bk, bv = (np.asarray(a, f32) for a in (bq, bk, bv))

    NQC, GQ = S // 512, 512

    def perm_w(a, nblk):  # (nblk*128, X) -> [128, nblk*X] partition-major
        X = a.shape[1]
        return np.ascontiguousarray(
            a.reshape(nblk, 128, X).transpose(1, 0, 2).reshape(128, nblk * X)
        ).astype(bf)

    def perm_hsT(hT):  # [D, S] -> [128, NQC*NDT*GQ] chunk-major
        t = hT.reshape(NDT, 128, NQC, GQ).transpose(1, 2, 0, 3)
        return np.ascontiguousarray(t.reshape(128, NQC * NDT * GQ)).astype(bf)

    hsT_b = [perm_hsT(hs[b].T) for b in range(B)]
    in_maps = []
    for c in range(NCORES):
        b, g = divmod(c, NCORES // B)
        cols = slice(g * GD, (g + 1) * GD)
        in_maps.append(
            {
                "hsT": hsT_b[b],
                "wq": perm_w(np.ascontiguousarray(Wq[:, cols]), NDT),
                "wk": perm_w(np.ascontiguousarray(Wk[:, cols]), NDT),
                "wv": perm_w(np.ascontiguousarray(Wv[:, cols]), NDT),
                "wo": perm_w(np.ascontiguousarray(Wo[cols, :]), 2),
                "bq": np.ascontiguousarray(bq[cols]),
                "bk": np.ascontiguousarray(bk[cols]),
                "bv": np.ascontiguousarray(np.broadcast_to(bv[cols], (128, GD))),
            }
        )
    return in_maps


_NC_CACHE: dict = {}


def _get_nc(s: int = S):
    if s not in _NC_CACHE:
        _NC_CACHE[s] = build_gau_nc(s)
    return _NC_CACHE[s]


def run_gau(in_maps, **kwargs):
    nc = _get_nc(S)
    return run_bass_kernel_spmd(nc, in_maps, core_ids=list(range(NCORES)), **kwargs)


def assemble_output(results, hidden_states, gating_factor, gating_bias, bo):
    """Sum per-batch head-group partials, transpose back, apply gating + bo.

    The per-(batch, query) gating scalar commutes through the AV matmul and
    the output projection, so it is applied here on the host.
    """
    bo = np.asarray(bo, np.float32)
    hs = np.asarray(hidden_states, np.float32)
    gf = np.float32(np.asarray(gating_factor, np.float32)[0])
    gb = np.float32(np.asarray(gating_bias, np.float32)[0])
    gate = 1.0 / (1.0 + np.exp(-(gf * hs.mean(axis=-1) + gb)))  # [B, S]
    gpb = NCORES // B
    out = np.empty((B, S, D), np.float32)
    MT, NQC, GQ = D // 128, S // 512, 512
    for b in range(B):
        acc = results[gpb * b]["outT"].astype(np.float32)
        for g in range(1, gpb):
            acc = acc + results[gpb * b + g]["outT"].astype(np.float32)
        # [128, NQC, MT, GQ] -> out^T [D, S]
        accT = acc.reshape(128, NQC, MT, GQ).transpose(2, 0, 1, 3).reshape(D, S)
        out[b] = accT.T * gate[b][:, None] + bo[None, :]
    return out


def kernel(hidden_states, Wq, bq, Wk, bk, Wv, bv, Wo, bo, gating_factor, gating_bias):
    in_maps = make_in_maps(
        hidden_states, Wq, bq, Wk, bk, Wv, bv, Wo, gating_factor, gating_bias
    )
    res = run_gau(in_maps)
    return assemble_output(res.results, hidden_states, gating_factor, gating_bias, bo)


# revision 42
# speedup vs baseline: 1.2576x; 1.0134x over previous
"""GAU attention (gated attention unit) Trainium2 Bass kernel.

Reference computation (B=2, S=2048, D=1024, H=16, DH=64):
    q = (hs @ Wq + bq), k = (hs @ Wk + bk), v = (hs @ Wv + bv)   per-head [B,S,H,DH]
    scores = q k^T / sqrt(DH);  probs = softmax(scores, axis=k)
    gating = sigmoid(gf * mean_d(hs) + gb)          # [B, S] per (batch, query)
    ctx = (probs * gating) @ v;  out = ctx @ Wo + bo

Sharding: 8 cores = 2 batches x 4 head-groups (4 heads each).  Each core
computes out^T partial [D, S] for its (batch, head-group); host sums the 4
partials per batch, applies the per-query gating scalar (gating commutes
through the linear AV + O-proj), and adds bo.

Per-core dataflow (matmuls bf16 with fp32 PSUM accumulation).  The kernel is
ACT(exp)-bound in steady state, so all projection work is interleaved into
the attention kt-loops to keep the exp pipeline dense:
  - hs^T [D,S] staged bf16 (host transposes + casts).
  - K^T [256,S]: chunk 0 of head-pair 0 projected up front; remaining chunks
    just-in-time inside the (qc0, pr) kt-loops.
  - V [S,256]: projected just-in-time inside the (qc0, pr0) kt-loop.
  - scores^T [k,q] per (pair, ktile, qchunk): two row-packed (tile_position
    (0,0)/(64,0)) K=64 matmuls -> exp -> E^T bf16.  Most tiles exp on ACT
    (scale=1/8); tiles in OFFLOAD_KT use a clamped bf16 exp bit-trick on DVE
    (two tensor_scalar ops) to relieve the ACT bottleneck.
  - softmax denom: DVE folds E^T ktiles into ks (first fold sums tiles 0+1),
    then ones[128,128] matmul broadcasts the 128-partition reduction.
  - AV: col-packed (tile_position (0,0)/(0,64)) matmuls, V stationary,
    E^T streaming -> ctx^T accumulated over ktiles in PSUM.
  - ctx^T scaled by 1/denom (DVE, bf16 out).  O-proj of chunk qc is
    interleaved into chunk qc+1's pr0 kt-loop; Q-proj of qc+1 into qc's pr1
    kt-loop, so ACT never waits on projection phases.
"""

import sys

for _p in ("/opt/trn_rl_repo", "/root/.axon_site/_ro/trn_rl_repo"):
    if _p not in sys.path:
        sys.path.append(_p)

import math
from contextlib import ExitStack

import ml_dtypes
import numpy as np

import concourse.bass as bass
import concourse.mybir as mybir
import concourse.tile as tile
from concourse import bacc
from concourse.bass_utils import run_bass_kernel_spmd

BF16 = mybir.dt.bfloat16
F32 = mybir.dt.float32
I16 = mybir.dt.int16
AF = mybir.ActivationFunctionType
OP = mybir.AluOpType

B, S, D, H = 2, 2048, 1024, 16
DH = 64
HPC = 4  # heads per core
GD = HPC * DH  # 256 (head-group width)
NCORES = 8
NDT = D // 128  # 8 contraction tiles over D

# E^T ktiles whose exp runs on DVE (bit-trick) instead of ACT
OFFLOAD_KT = ()
# bf16 Schraudolph constants: i16 = min(max(s*EXA, EXLO), EXHI) + EXB;
# E = bitcast_bf16(i16) ~= exp(s/8), max rel err ~4%, zero-mean.
EXA = 0.125 * 128.0 / math.log(2.0)
EXB = 16256.0 - 486411.0 / 65536.0 + 0.5
EXLO = 128.0 - 16249.0
EXHI = 18304.0 - 16249.0


def _build(ctx: ExitStack, tc: "tile.TileContext", io: dict, s: int):
    nc = tc.nc
    GQ = min(512, s)
    NQC = s // GQ  # q chunks
    NKT = s // 128  # k tiles

    hsT, wq, wk, wv, wo = io["hsT"], io["wq"], io["wk"], io["wv"], io["wo"]
    bq, bk, bv, outT = io["bq"], io["bk"], io["bv"], io["outT"]

    consts = ctx.enter_context(tc.tile_pool(name="consts", bufs=1))
    sb = ctx.enter_context(tc.tile_pool(name="sb", bufs=1))
    etp = ctx.enter_context(tc.tile_pool(name="etp", bufs=6))
    ksp = ctx.enter_context(tc.tile_pool(name="ksp", bufs=2))
    outp = ctx.enter_context(tc.tile_pool(name="outp", bufs=2))
    # PSUM budget: 2x2 (scores, 2-bank tiles) + 2 (ctx) + 2 (proj/denom/
    # o-proj, shared tag) = 8 banks
    ps_mm = ctx.enter_context(tc.tile_pool(name="ps_mm", bufs=2, space="PSUM"))
    ps_ctx = ctx.enter_context(tc.tile_pool(name="ps_ctx", bufs=2, space="PSUM"))
    ps_o = ctx.enter_context(tc.tile_pool(name="ps_o", bufs=2, space="PSUM"))

    # ---- constants ----
    ones128 = consts.tile([128, 128], BF16, tag="ones128", name="ones128")
    nc.vector.memset(ones128[:], 1.0)

    bq_sb = consts.tile([128, 2], F32, tag="bq", name="bq")
    bk_sb = consts.tile([128, 2], F32, tag="bk", name="bk")
    zbias = consts.tile([128, 1], F32, tag="zbias", name="zbias")
    nc.vector.memset(zbias[:], 0.0)
    bv_bc = consts.tile([128, GD], F32, tag="bvbc", name="bvbc")

    # ---- load weights FIRST (small; K/Q-proj need them before attention
    # can start), then hs^T chunk by chunk.  All tensors arrive host-permuted
    # to partition-major layouts so every DMA is one contiguous run per
    # partition (128 descriptors instead of ~1024): cuts both the serial
    # per-dma sequencer issue cost and descriptor-processing time. ----
    wq_sb = consts.tile([128, NDT, GD], BF16, tag="wq", name="wq")
    wk_sb = consts.tile([128, NDT, GD], BF16, tag="wk", name="wk")
    wv_sb = consts.tile([128, NDT, GD], BF16, tag="wv", name="wv")
    wo_sb = consts.tile([128, 2, D], BF16, tag="wo", name="wo")
    # hs^T chunk-major [p, chunk, dtile, col] so per-chunk DMAs are contiguous
    hsT_sb = sb.tile([128, NQC, NDT, GQ], BF16, tag="hsT", name="hsT")
    CW = NDT * GQ  # host-layout elements per chunk per partition

    # PE warmup: throwaway matmuls during the input-DMA wait keep the PE
    # busy so the HAM clock gate releases (1.2 -> 2.4 GHz) before the first
    # real projection instead of ~3.4us into it.
    warm = ps_o.tile([128, GQ], F32, tag="po", name="warm")
    for _ in range(72):
        nc.tensor.matmul(warm[:, 0:128], lhsT=ones128[:], rhs=ones128[:], start=True, stop=True)

    nc.sync.dma_start(wk_sb[:], wk[:, :])
    # chunk 0 in two halves so K-proj's first d-tiles start sooner
    nc.sync.dma_start(hsT_sb[:, 0, 0 : NDT // 2, :], hsT[:, 0 : CW // 2])
    nc.sync.dma_start(wq_sb[:], wq[:, :])
    nc.sync.dma_start(hsT_sb[:, 0, NDT // 2 :, :], hsT[:, CW // 2 : CW])
    nc.sync.dma_start(wv_sb[:], wv[:, :])
    nc.sync.dma_start(wo_sb[:], wo[:, :])
    nc.sync.dma_start(bq_sb[:], bq.rearrange("(m p) -> p m", p=128))
    nc.sync.dma_start(bk_sb[:], bk.rearrange("(m p) -> p m", p=128))
    nc.sync.dma_start(bv_bc[:], bv[:, :])
    for qc in range(1, NQC):
        nc.sync.dma_start(hsT_sb[:, qc, :, :], hsT[:, qc * CW : (qc + 1) * CW])

    qT_sb = [sb.tile([128, s], BF16, tag=f"qT{m}", name=f"qT{m}") for m in range(2)]
    kT_sb = [sb.tile([128, s], BF16, tag=f"kT{m}", name=f"kT{m}") for m in range(2)]
    v_sb = [sb.tile([128, GD], BF16, tag=f"v{st}", name=f"v{st}") for st in range(NKT)]

    def kproj(m, c):
        ms = slice(m * 128, (m + 1) * 128)
        cls = slice(c * GQ, (c + 1) * GQ)
        p = ps_o.tile([128, GQ], F32, tag="po", name="kp")
        for d in range(NDT):
            nc.tensor.matmul(
                p[:], lhsT=wk_sb[:, d, ms], rhs=hsT_sb[:, c, d, :],
                start=(d == 0), stop=(d == NDT - 1),
            )
        nc.vector.tensor_scalar_add(kT_sb[m][:, cls], p[:], bk_sb[:, m : m + 1])

    def qproj(m, c):
        ms = slice(m * 128, (m + 1) * 128)
        cls = slice(c * GQ, (c + 1) * GQ)
        p = ps_o.tile([128, GQ], F32, tag="po", name="qp")
        for d in range(NDT):
            nc.tensor.matmul(
                p[:], lhsT=wq_sb[:, d, ms], rhs=hsT_sb[:, c, d, :],
                start=(d == 0), stop=(d == NDT - 1),
            )
        nc.vector.tensor_scalar_add(qT_sb[m][:, cls], p[:], bq_sb[:, m : m + 1])

    def vproj(st):
        sc_, so_ = st // 4, (st % 4) * 128
        vp = ps_o.tile([128, GD], F32, tag="po", name="vp")
        for d in range(NDT):
            nc.tensor.matmul(
                vp[:], lhsT=hsT_sb[:, sc_, d, so_ : so_ + 128], rhs=wv_sb[:, d, :],
                start=(d == 0), stop=(d == NDT - 1),
            )
        nc.vector.tensor_tensor(v_sb[st][:], vp[:], bv_bc[:], op=OP.add)

    ctx_sc_of = {}
    ost_of = {}
    OW = (D // 128) * GQ  # out elements per chunk per partition (host layout)

    def oproj(qc, mt):
        cs = slice(qc * GQ, (qc + 1) * GQ)
        ms = slice(mt * 128, (mt + 1) * 128)
        o_ps = ps_o.tile([128, GQ], F32, tag="po", name="po")
        for pr in range(2):
            nc.tensor.matmul(
                o_ps[:], lhsT=wo_sb[:, pr, ms], rhs=ctx_sc_of[qc][pr][:],
                start=(pr == 0), stop=(pr == 1),
            )
        if mt == 0:
            ost_of[qc] = outp.tile([128, D // 128, GQ], BF16, tag="ost", name="ost")
        ost = ost_of[qc]
        nc.vector.tensor_copy(ost[:, mt, :], o_ps[:])
        # split out-DMAs so later copies overlap earlier transfers; the final
        # chunk (the kernel tail) drains in quarters
        half = D // 256
        if qc == NQC - 1:
            if mt % 2 == 1:
                lo = qc * OW + (mt - 1) * GQ
                nc.sync.dma_start(outT[:, lo : lo + 2 * GQ], ost[:, mt - 1 : mt + 1, :])
        elif mt == half - 1:
            nc.sync.dma_start(outT[:, qc * OW : qc * OW + OW // 2], ost[:, 0:half, :])
        elif mt == D // 128 - 1:
            nc.sync.dma_start(outT[:, qc * OW + OW // 2 : (qc + 1) * OW], ost[:, half:, :])

    # serial head: K^T chunk 0 (pair 0) and Q^T chunk 0, so scores start ASAP
    kproj(0, 0)
    qproj(0, 0)
    qproj(1, 0)

    # interleave schedule: (qc, pr) -> {kt: [work closures]}
    def make_sched():
        sched = {(qc, pr): {} for qc in range(NQC) for pr in range(2)}

        def add(qc, pr, kt, fn):
            sched[(qc, pr)].setdefault(kt, []).append(fn)

        for st in range(NKT):  # V just-in-time in (qc0, pr0)
            add(0, 0, st, lambda st=st: vproj(st))
        for c in range(1, NQC):  # K chunks 1..3, pair 0
            add(0, 0, 4 * c - 3, lambda c=c: kproj(0, c))
        add(0, 0, NKT - 3, lambda: kproj(1, 0))  # K chunk 0, pair 1
        for c in range(1, NQC):  # K chunks 1..3, pair 1
            add(0, 1, 4 * c - 3, lambda c=c: kproj(1, c))
        for qc in range(NQC - 1):  # Q-proj of qc+1 late in (qc, pr1)
            add(qc, 1, NKT - 5, lambda qc=qc: qproj(0, qc + 1))
            add(qc, 1, NKT - 3, lambda qc=qc: qproj(1, qc + 1))
        for qc in range(1, NQC):  # O-proj of qc-1 split across (qc, pr0/pr1)
            for mt in range(D // 128):
                add(qc, mt // 4, mt % 4 + 1, lambda qc=qc, mt=mt: oproj(qc - 1, mt))
        return sched

    sched = make_sched()

    # ---- per q-chunk: attention (projections interleaved per sched) ----
    pending_fin = [None]  # deferred finalize of the previous (qc, pr) window

    for qc in range(NQC):
        cs = slice(qc * GQ, (qc + 1) * GQ)
        ctx_sc = [sb.tile([128, GQ], BF16, tag=f"ctxs{p}_{qc % 2}", name=f"ctxs{p}_{qc % 2}") for p in range(2)]
        ctx_sc_of[qc] = ctx_sc
        for pr in range(2):
            # one PSUM bank per head: col-packed AV matmuls write disjoint
            # partition ranges, but each bank hosts a single accumulation group
            ctx_psA = ps_ctx.tile([128, GQ], F32, tag="ctx", name="ctxA")
            ctx_psB = ps_ctx.tile([128, GQ], F32, tag="ctx", name="ctxB")
            ks = ksp.tile([128, 2 * GQ], BF16, tag="ks", name="ks")
            # Software-pipelined over kt: the AV pair for kt-1 is emitted right
            # after the scores pair for kt; exp/fold of kt overlap the next
            # group's matmuls; interleaved projection work fills PE slack.
            ets = [None] * NKT
            for kt in range(NKT + 1):
                if kt < NKT:
                    ks_ = slice(kt * 128, (kt + 1) * 128)
                    # heads A|B share one 2-bank psum tile -> single wide exp
                    sp = ps_mm.tile([128, 2 * GQ], F32, tag="smm", name="smm")
                    nc.tensor.matmul(
                        sp[:, 0:GQ], lhsT=kT_sb[pr][0:64, ks_], rhs=qT_sb[pr][0:64, cs],
                        tile_position=(0, 0), start=True, stop=True,
                    )
                    nc.tensor.matmul(
                        sp[:, GQ : 2 * GQ], lhsT=kT_sb[pr][64:128, ks_], rhs=qT_sb[pr][64:128, cs],
                        tile_position=(64, 0), start=True, stop=True,
                    )
                    # finalize of the previous window lands AFTER this window's
                    # first two score groups are in flight, so the PE chews new
                    # scores while the old exp->fold->denominator chain drains.
                    # It must precede the interleaved sched work (oproj reads
                    # ctx_sc of the window being finalized).
                    if kt == 1 and pending_fin[0] is not None:
                        pending_fin[0]()
                        pending_fin[0] = None
                    for fn in sched[(qc, pr)].get(kt, ()):
                        fn()
                if kt > 0:
                    pv = kt - 1
                    et = ets[pv]
                    nc.tensor.matmul(
                        ctx_psA[0:64, :], lhsT=v_sb[pv][:, pr * 128 : pr * 128 + 64],
                        rhs=et[:, 0:GQ], tile_position=(0, 0),
                        start=(pv == 0), stop=(pv == NKT - 1),
                    )
                    nc.tensor.matmul(
                        ctx_psB[64:128, :], lhsT=v_sb[pv][:, pr * 128 + 64 : pr * 128 + 128],
                        rhs=et[:, GQ : 2 * GQ], tile_position=(0, 64),
                        start=(pv == 0), stop=(pv == NKT - 1),
                    )
                if kt < NKT:
                    et = etp.tile([128, 2 * GQ], BF16, tag="et", name="et")
                    ets[kt] = et
                    if kt in OFFLOAD_KT:
                        # clamped bf16 exp bit-trick on DVE (2 tensor_scalar ops)
                        u = ksp.tile([128, 2 * GQ], F32, tag="shru", name="shru")
                        nc.vector.tensor_scalar(u[:], sp[:], EXA, EXLO, op0=OP.mult, op1=OP.max)
                        nc.vector.tensor_scalar(
                            et[:].bitcast(I16), u[:], EXHI, EXB, op0=OP.min, op1=OP.add
                        )
                    else:
                        # single wide exp over both heads (2-bank PSUM read)
                        nc.scalar.activation(et[:], sp[:], AF.Exp, bias=zbias[:, 0:1], scale=0.125)
                    if kt == 1:
                        nc.vector.tensor_tensor(ks[:], ets[0][:], ets[1][:], op=OP.add)
                    elif kt > 1:
                        nc.vector.tensor_tensor(ks[:], ks[:], et[:], op=OP.add)
            def finalize(ks=ks, ctx_psA=ctx_psA, ctx_psB=ctx_psB, dst=ctx_sc[pr]):
                # softmax denominators (broadcast to all partitions) and ctx
                # scaling.  Head A's recip+scale run first so its ctx psum
                # bank frees as early as possible for the next window's AV.
                rA = ksp.tile([128, GQ], F32, tag="rrA", name="rrA")
                rB = ksp.tile([128, GQ], F32, tag="rrB", name="rrB")
                dbA = ps_o.tile([128, GQ], F32, tag="po", name="po")
                nc.tensor.matmul(dbA[:], lhsT=ones128[:], rhs=ks[:, 0:GQ], start=True, stop=True)
                dbB = ps_o.tile([128, GQ], F32, tag="po", name="po")
                nc.tensor.matmul(dbB[:], lhsT=ones128[:], rhs=ks[:, GQ : 2 * GQ], start=True, stop=True)
                nc.vector.reciprocal_approx_fast(rA[:], dbA[:])
                nc.vector.tensor_tensor(dst[0:64, :], ctx_psA[0:64, :], rA[0:64, :], op=OP.mult)
                nc.vector.reciprocal_approx_fast(rB[:], dbB[:])
                nc.vector.tensor_tensor(dst[64:128, :], ctx_psB[64:128, :], rB[64:128, :], op=OP.mult)

            pending_fin[0] = finalize

    pending_fin[0]()
    pending_fin[0] = None

    # tail: O-projection of the last chunk
    for mt in range(D // 128):
        oproj(NQC - 1, mt)


def build_gau_nc(s: int = S, debug: bool = False):
    nc = bacc.Bacc("TRN2", target_bir_lowering=False, debug=debug, num_devices=NCORES)
    io = {
        "hsT": nc.dram_tensor("hsT", [128, NDT * s], BF16, kind="ExternalInput").ap(),
        "wq": nc.dram_tensor("wq", [128, NDT * GD], BF16, kind="ExternalInput").ap(),
        "wk": nc.dram_tensor("wk", [128, NDT * GD], BF16, kind="ExternalInput").ap(),
        "wv": nc.dram_tensor("wv", [128, NDT * GD], BF16, kind="ExternalInput").ap(),
        "wo": nc.dram_tensor("wo", [128, 2 * D], BF16, kind="ExternalInput").ap(),
        "bq": nc.dram_tensor("bq", [GD], F32, kind="ExternalInput").ap(),
        "bk": nc.dram_tensor("bk", [GD], F32, kind="ExternalInput").ap(),
        "bv": nc.dram_tensor("bv", [128, GD], F32, kind="ExternalInput").ap(),
        "outT": nc.dram_tensor("outT", [128, (D // 128) * s], BF16, kind="ExternalOutput").ap(),
    }
    with tile.TileContext(nc) as tc:
        with ExitStack() as ctx:
            _build(ctx, tc, io, s)
    nc.compile()
    return nc


def make_in_maps(hidden_states, Wq, bq, Wk, bk, Wv, bv, Wo, gating_factor, gating_bias):
    """Shard full inputs into 8 per-core input maps (host-side prep)."""
    bf = ml_dtypes.bfloat16
    f32 = np.float32
    hs = np.asarray(hidden_states, f32)
    Wq, Wk, Wv, Wo = (np.asarray(a, f32) for a in (Wq, Wk, Wv, Wo))
    bq, bk, bv = (np.asarray(a, f32) for a in (bq, bk, bv))

    NQC, GQ = S // 512, 512

    def perm_w(a, nblk):  # (nblk*128, X) -> [128, nblk*X] partition-major
        X = a.shape[1]
        return np.ascontiguousarray(
            a.reshape(nblk, 128, X).transpose(1, 0, 2).reshape(128, nblk * X)
        ).astype(bf)

    def perm_hsT(hT):  # [D, S] -> [128, NQC*NDT*GQ] chunk-major
        t = hT.reshape(NDT, 128, NQC, GQ).transpose(1, 2, 0, 3)
        return np.ascontiguousarray(t.reshape(128, NQC * NDT * GQ)).astype(bf)

    hsT_b = [perm_hsT(hs[b].T) for b in range(B)]
    in_maps = []
    for c in range(NCORES):
        b, g = divmod(c, NCORES // B)
        cols = slice(g * GD, (g + 1) * GD)
        in_maps.append(
            {
                "hsT": hsT_b[b],
                "wq": perm_w(np.ascontiguousarray(Wq[:, cols]), NDT),
                "wk": perm_w(np.ascontiguousarray(Wk[:, cols]), NDT),
                "wv": perm_w(np.ascontiguousarray(Wv[:, cols]), NDT),
                "wo": perm_w(np.ascontiguousarray(Wo[cols, :]), 2),
                "bq": np.ascontiguousarray(bq[cols]),
                "bk": np.ascontiguousarray(bk[cols]),
                "bv": np.ascontiguousarray(np.broadcast_to(bv[cols], (128, GD))),
            }
        )
    return in_maps


_NC_CACHE: dict = {}


def _get_nc(s: int = S):
    if s not in _NC_CACHE:
        _NC_CACHE[s] = build_gau_nc(s)
    return _NC_CACHE[s]


def run_gau(in_maps, **kwargs):
    nc = _get_nc(S)
    return run_bass_kernel_spmd(nc, in_maps, core_ids=list(range(NCORES)), **kwargs)


def assemble_output(results, hidden_states, gating_factor, gating_bias, bo):
    """Sum per-batch head-group partials, transpose back, apply gating + bo.

    The per-(batch, query) gating scalar commutes through the AV matmul and
    the output projection, so it is applied here on the host.
    """
    bo = np.asarray(bo, np.float32)
    hs = np.asarray(hidden_states, np.float32)
    gf = np.float32(np.asarray(gating_factor, np.float32)[0])
    gb = np.float32(np.asarray(gating_bias, np.float32)[0])
    gate = 1.0 / (1.0 + np.exp(-(gf * hs.mean(axis=-1) + gb)))  # [B, S]
    gpb = NCORES // B
    out = np.empty((B, S, D), np.float32)
    MT, NQC, GQ = D // 128, S // 512, 512
    for b in range(B):
        acc = results[gpb * b]["outT"].astype(np.float32)
        for g in range(1, gpb):
            acc = acc + results[gpb * b + g]["outT"].astype(np.float32)
        # [128, NQC, MT, GQ] -> out^T [D, S]
        accT = acc.reshape(128, NQC, MT, GQ).transpose(2, 0, 1, 3).reshape(D, S)
        out[b] = accT.T * gate[b][:, None] + bo[None, :]
    return out


def kernel(hidden_states, Wq, bq, Wk, bk, Wv, bv, Wo, bo, gating_factor, gating_bias):
    in_maps = make_in_maps(
        hidden_states, Wq, bq, Wk, bk, Wv, bv, Wo, gating_factor, gating_bias
    )
    res = run_gau(in_maps)
    return assemble_output(res.results, hidden_states, gating_factor, gating_bias, bo)
